# revision 33
# baseline (speedup 1.0000x reference)
"""Distributed Trainium2 kernel for nn_Attention_65764539236808.

Multi-head causal self-attention layer (SEQ=2048, BATCH=2, HIDDEN=2048,
HEADS=16, HEAD_DIM=128) on 8 NeuronCores, tensor-parallel over heads
(2 heads/core).

v3 design (pipelined collectives; PE-roofline oriented):
  - 8 sections, one per 512-token block tb: QKV projection for tb
    interleaved with BOTH local heads' attention block (b, qb) =
    (tb//4, tb%4), which only needs K/V tiles produced by this and
    earlier sections.  So all of batch-0's ctx is finished at
    mid-kernel and batch-1's ctx finishes in two half-batch waves.
  - Three AllToAlls, each fully (or mostly) hidden:
      a2a#A  (b0 ctx, 1MB)   triggers inside section tb=4, covered by
                             sections 4-7 (~140us of PE work);
      a2a#B1 (b1 qb0-1, .5MB) triggers inside section tb=6;
      a2a#B2 (b1 qb2-3, .5MB) triggers right after section tb=7,
                             covered by dense pass A (~34us).
  - Token ownership per core c: 256 b0 tokens [256c,256c+256), plus
    128+128 b1 tokens [128c,+128) in each half of batch 1.  The dense
    projection runs as two disjoint-token passes (A: b0 cols 0:256,
    B: b1 cols 256:512), each applying the bias — no partial-sum
    buffer and no second all-reduce.
  Attention block internals (from v2):
  - scores^T layout [sk, sq]; diagonal 128x512 key-tiles only compute
    the un-masked column range (N = 512-128*kd) and run FIRST so their
    exp+mask chains finish before their ctx matmuls come up.
  - softmax denominator: e-tiles are accumulated into E_total on
    VectorE (f32); a single ones-matmul per block reduces over
    partitions in the epilogue ("a-mode" everywhere — the PE hybrid
    row-sum of v2 is gone, saving ~29k PE cycles, since every a2a
    trigger chain now has PE cover).
  - 1/Z via ScalarE exp(-ln(Z)); b_v folded into the dense bias on the
    host (sum(probs)==1).
  PSUM: one shared [128,1024] ring (2 bufs, 4 banks) carries qk-pairs
  and score pairs; a [128,512] ring (4 bufs) holds v-groups, ctx
  accumulators, Z and dense outputs.
"""

import math
import os
import sys
import types

import numpy as np
import ml_dtypes

import concourse.bass as bass
import concourse.mybir as mybir
import concourse.tile as tile
from concourse.bass import ts, ds
from concourse.bass_utils import run_bass_kernel_spmd

try:
    import orjson as _json_mod

    def _jloads(b):
        return _json_mod.loads(b)

    def _jdumps(o):
        return _json_mod.dumps(o)
except ImportError:  # pragma: no cover
    import json as _json_mod

    def _jloads(b):
        return _json_mod.loads(b)

    def _jdumps(o):
        return _json_mod.dumps(o).encode()

N_CORES = 8
SEQ, BATCH, HIDDEN, HEADS = 2048, 2, 2048, 16
HD = HIDDEN // HEADS          # 128
T = SEQ * BATCH               # 4096 tokens, batch-major: t = b*SEQ + s
P = 128
TBLK = 512                    # token block (free-dim tile)
NTB = T // TBLK               # 8
KO = HIDDEN // P              # 16 k-tiles over hidden
TOK_SHARD = T // N_CORES      # 512 tokens per core for the output
B0_SH = 256                   # per-core b0 token shard
B1_SH = 128                   # per-core b1 token shard per half
SCALE = 1.0 / math.sqrt(HD)

BF16 = mybir.dt.bfloat16
F32 = mybir.dt.float32

_last_exec_time_ns = None
_last_res = None


# ----------------------------------------------------------------------------
# Workaround: this walrus build accepts only ONE sync-wait per instruction.
# Hoist extra on_wait entries onto single-wait EventSemaphore instructions
# inserted just before the owner (same engine => same program order, so the
# semantics are identical).
# ----------------------------------------------------------------------------
def _split_multiwait(bir: dict) -> dict:
    ctr = 0
    for fn in bir.get("functions", []):
        for blk in fn.get("blocks", []):
            insts = blk.get("instructions")
            if not insts:
                continue
            new_insts = []
            changed = False
            for inst in insts:
                si = inst.get("sync_info")
                ow = (si or {}).get("on_wait") or []
                if len(ow) > 1:
                    changed = True
                    for w in ow[:-1]:
                        ctr += 1
                        new_insts.append(
                            {
                                "debug": inst.get("debug", 0),
                                "engine": inst["engine"],
                                "ins": [],
                                "name": f"{inst['name']}-mw{ctr}",
                                "opcode": "EventSemaphore",
                                "outs": [],
                                "sync_info": {"on_update": [], "on_wait": [w]},
                            }
                        )
                    si["on_wait"] = [ow[-1]]
                new_insts.append(inst)
            if changed:
                blk["instructions"] = new_insts
    return bir


def _patch_bass(nc):
    if getattr(nc, "_waitfix_patched", False):
        return nc
    orig = nc.to_json_bytes

    def patched():
        return _jdumps(_split_multiwait(_jloads(orig())))

    nc.to_json_bytes = patched
    nc._waitfix_patched = True
    return nc


def _install_ntff_hook():
    """Recreate antenv.axon_hooks if the image lacks it (needed for trace=True)."""
    try:
        from antenv.axon_hooks import get_axon_ntff_profile_hook  # noqa: F401
        return True
    except ImportError:
        pass
    try:
        from trn_agent_boot.trn_boot import _ntff_profile_via_ctypes

        hook = _ntff_profile_via_ctypes("/opt/axon/libaxon_pjrt.so")
        if hook is None:
            return False
        mod = types.ModuleType("antenv.axon_hooks")
        mod._hook = hook
        mod.get_axon_ntff_profile_hook = lambda: mod._hook
        mod.set_axon_ntff_profile_hook = lambda h: setattr(mod, "_hook", h)
        sys.modules["antenv.axon_hooks"] = mod
        import antenv

        antenv.axon_hooks = mod
        return True
    except Exception:
        return False


# ----------------------------------------------------------------------------
# Device graph (SPMD: same graph on all 8 cores)
# ----------------------------------------------------------------------------
def _build():
    nc = bass.Bass()

    xT = nc.declare_dram_parameter("xT", [HIDDEN, T], BF16, isOutput=False)
    wqk = nc.declare_dram_parameter("wqk", [HIDDEN, 4 * P], BF16, isOutput=False)
    wv = nc.declare_dram_parameter("wv", [HIDDEN, 2 * P], BF16, isOutput=False)
    # wd pre-tiled on the host as [ot, p, ki, ocol] so each per-ot load is
    # one contiguous 4KB-per-partition DMA
    wd = nc.declare_dram_parameter("wd", [KO, P, KO, P], BF16, isOutput=False)
    bqk = nc.declare_dram_parameter("bqk", [P, 4], F32, isOutput=False)
    bd = nc.declare_dram_parameter("bd", [P, KO], F32, isOutput=False)  # + Wd@bv
    out = nc.declare_dram_parameter("out", [HIDDEN, TOK_SHARD], F32, isOutput=True)

    xT_r = xT.rearrange("(ko p) t -> p ko t", p=P)
    wqk_r = wqk.rearrange("(ko p) c -> p ko c", p=P)
    wv_r = wv.rearrange("(ko p) c -> p ko c", p=P)

    Exp = mybir.ActivationFunctionType.Exp
    Ln = mybir.ActivationFunctionType.Ln
    Ident = mybir.ActivationFunctionType.Identity

    with tile.TileContext(nc) as tc:
        with (
            tc.tile_pool(name="const", bufs=1) as pc,
            tc.tile_pool(name="xs", bufs=2) as px,
            tc.tile_pool(name="es", bufs=14) as pe,
            tc.tile_pool(name="esum", bufs=3) as pE,
            tc.tile_pool(name="fs", bufs=3) as pf,
            tc.tile_pool(name="wds", bufs=6) as pwd,
            tc.tile_pool(name="outs", bufs=3) as po,
            tc.tile_pool(name="ps_ring", bufs=2, space="PSUM") as pps,
            tc.tile_pool(name="ps_acc", bufs=4, space="PSUM") as pacc,
            tc.tile_pool(name="dram", bufs=1, space="DRAM") as pdram,
        ):
            # ---- constants ----
            # preamble rides three queues in parallel: wqk on sync, x0 on
            # scalar, wv on gpsimd — the tb0 ko-major qk loop consumes
            # wqk[ko]+x0[ko] at ~250GB/s aggregate, which one queue alone
            # cannot sustain.
            wqk_sb = pc.tile([P, KO, 4 * P], BF16)
            x0_sb = px.tile([P, KO, TBLK], BF16, tag="x")
            for lo, n in [(0, 1), (1, 1), (2, 1), (3, 1), (4, 2), (6, 2),
                          (8, 4), (12, 4)]:
                nc.sync.dma_start(
                    wqk_sb[:, ds(lo, n), :], wqk_r[:, ds(lo, n), :]
                )
                nc.sync.dma_start(
                    x0_sb[:, ds(lo, n), :], xT_r[:, ds(lo, n), ts(0, TBLK)]
                )
            wv_sb = pc.tile([P, KO, 2 * P], BF16)
            for ko4 in range(4):
                nc.gpsimd.dma_start(
                    wv_sb[:, ts(ko4, 4), :], wv_r[:, ts(ko4, 4), :]
                )
            bqk_sb = pc.tile([P, 4], F32)
            nc.sync.dma_start(bqk_sb[:], bqk[:])
            bd_sb = pc.tile([P, KO], F32)
            nc.sync.dma_start(bd_sb[:], bd[:])

            ones_sb = pc.tile([P, P], BF16)
            nc.vector.memset(ones_sb[:], 1.0)
            # triangular mask [sk, sq_local]: keep where sq >= sk
            tri_sb = pc.tile([P, P], BF16)
            nc.vector.memset(tri_sb[:], 1.0)
            nc.gpsimd.affine_select(
                out=tri_sb[:],
                in_=tri_sb[:],
                compare_op=mybir.AluOpType.is_ge,
                fill=0.0,
                base=0,
                pattern=[[1, P]],
                channel_multiplier=-1,
            )

            qk_sb = pc.tile([P, 4, T], BF16)   # [d, (q_h0,k_h0,q_h1,k_h1), tok]
            v_sb = pc.tile([P, T // P, 2 * P], BF16)  # [tok_in_tile, tile, (v0,v1)]

            # a2a staging: A carries b0 ctx (dst shard 256 tokens),
            # B1/B2 carry one half of b1 each (dst shard 128 tokens).
            a2a_inA = pdram.tile([N_CORES, 2, P, B0_SH], BF16,
                                 name="a2a_inA", tag="a2a_inA")
            a2a_outA = pdram.tile([N_CORES, 2, P, B0_SH], BF16,
                                  name="a2a_outA", tag="a2a_outA")
            # B1 carries b1 query-blocks 0-2 (192-token shards), B2 only
            # block 3 (64-token shards) so the last collective is tiny and
            # triggers with almost no attention left in front of it.
            B1W, B2W = 192, 64
            a2a_inB = [
                pdram.tile([N_CORES, 2, P, w], BF16,
                           name=f"a2a_inB{k}", tag=f"a2a_inB{k}")
                for k, w in ((0, B1W), (1, B2W))
            ]
            a2a_outB = [
                pdram.tile([N_CORES, 2, P, w], BF16,
                           name=f"a2a_outB{k}", tag=f"a2a_outB{k}")
                for k, w in ((0, B1W), (1, B2W))
            ]
            # [d, global head, tok] gathered ctx for this core's shards
            ctxT_A = pc.tile([P, HEADS, B0_SH], BF16, name="ctxT_A")
            ctxT_B = pc.tile([P, HEADS, 2 * B1_SH], BF16, name="ctxT_B")

            # collective triggers ride gpsimd (and ONLY the triggers: the
            # gpsimd SWDGE costs ~3.3us per dma op, serially, so any loads
            # placed there both run late and block later triggers).  The
            # gathered-ctx loads ride the sync HWDGE queue instead, emitted
            # at a point where their gating collective is already complete
            # so they never head-of-line-block the x stream.
            def emit_a2a(which):
                if which == "A":
                    nc.gpsimd.collective_compute(
                        "AllToAll",
                        mybir.AluOpType.bypass,
                        replica_groups=[list(range(N_CORES))],
                        ins=[a2a_inA[:].opt()],
                        outs=[a2a_outA[:].opt()],
                    )
                else:
                    k = int(which)
                    nc.gpsimd.collective_compute(
                        "AllToAll",
                        mybir.AluOpType.bypass,
                        replica_groups=[list(range(N_CORES))],
                        ins=[a2a_inB[k][:].opt()],
                        outs=[a2a_outB[k][:].opt()],
                    )

            def emit_ctx_loads(which):
                if which == "A":
                    for i in range(N_CORES):
                        for h in range(2):
                            nc.sync.dma_start(
                                ctxT_A[:, 2 * i + h, :], a2a_outA[i, h, :, :]
                            )
                else:
                    k = int(which)
                    c0 = 0 if k == 0 else B1W
                    w = B1W if k == 0 else B2W
                    for i in range(N_CORES):
                        for h in range(2):
                            nc.sync.dma_start(
                                ctxT_B[:, 2 * i + h, ds(c0, w)],
                                a2a_outB[k][i, h, :, :],
                            )

            # ------------------------------------------------------------
            # attention block emission.  Returns (s_units, c_units, state):
            # score units (diag packs first, then off-diag pairs, then the
            # E finisher) and ctx-matmul units.  Row sums accumulate into
            # E on the DVE ("a-mode"); the Z ones-matmul runs in the
            # epilogue.
            # ------------------------------------------------------------
            def make_block(h, b, qb):
                base = b * SEQ
                q0 = base + qb * TBLK
                st = {}

                def k_ap(kt):
                    return qk_sb[:, 2 * h + 1, ds(base + kt * P, P)]

                s_units = []
                c_units = []

                def sA():
                    st["Et"] = pE.tile([P, TBLK], F32, tag="Ef", name="Et")
                    psA = pps.tile([P, 2 * TBLK], F32, tag="ps", name="psA")
                    st["psA"] = psA
                    nc.tensor.matmul(
                        psA[:, 0:TBLK], lhsT=k_ap(4 * qb + 0),
                        rhs=qk_sb[:, 2 * h, ds(q0, TBLK)], start=True, stop=True,
                    )
                    nc.tensor.matmul(
                        psA[:, ds(TBLK, 384)], lhsT=k_ap(4 * qb + 1),
                        rhs=qk_sb[:, 2 * h, ds(q0 + P, 384)], start=True, stop=True,
                    )
                s_units.append(sA)

                def sAe():
                    eA = pe.tile([P, 2 * TBLK], BF16, tag="e", name="eA")
                    st["eA"] = eA
                    nc.scalar.activation(
                        eA[:, 0:TBLK + 384], st["psA"][:, 0:TBLK + 384],
                        Exp, scale=SCALE,
                    )
                    nc.vector.tensor_mul(eA[:, 0:P], eA[:, 0:P], tri_sb[:])
                    nc.vector.tensor_mul(
                        eA[:, ds(TBLK, P)], eA[:, ds(TBLK, P)], tri_sb[:]
                    )
                    if qb == 0:
                        nc.vector.tensor_copy(st["Et"][:], eA[:, 0:TBLK])
                s_units.append(sAe)

                def sB():
                    psB = pps.tile([P, 2 * TBLK], F32, tag="ps", name="psB")
                    st["psB"] = psB
                    nc.tensor.matmul(
                        psB[:, 0:256], lhsT=k_ap(4 * qb + 2),
                        rhs=qk_sb[:, 2 * h, ds(q0 + 2 * P, 256)],
                        start=True, stop=True,
                    )
                    nc.tensor.matmul(
                        psB[:, ds(TBLK, P)], lhsT=k_ap(4 * qb + 3),
                        rhs=qk_sb[:, 2 * h, ds(q0 + 3 * P, P)],
                        start=True, stop=True,
                    )
                s_units.append(sB)

                def sBe():
                    eB = pe.tile([P, 2 * TBLK], BF16, tag="e", name="eB")
                    st["eB"] = eB
                    nc.scalar.activation(
                        eB[:, 0:TBLK + P], st["psB"][:, 0:TBLK + P],
                        Exp, scale=SCALE,
                    )
                    nc.vector.tensor_mul(eB[:, 0:P], eB[:, 0:P], tri_sb[:])
                    nc.vector.tensor_mul(
                        eB[:, ds(TBLK, P)], eB[:, ds(TBLK, P)], tri_sb[:]
                    )
                s_units.append(sBe)

                for pr in range(2 * qb):
                    def s_od(pr=pr):
                        ps_s = pps.tile([P, 2 * TBLK], F32, tag="ps", name="ps_s")
                        for half in range(2):
                            kt = 2 * pr + half
                            nc.tensor.matmul(
                                ps_s[:, ts(half, TBLK)],
                                lhsT=k_ap(kt),
                                rhs=qk_sb[:, 2 * h, ds(q0, TBLK)],
                                start=True, stop=True,
                            )
                        e_od = pe.tile([P, 2 * TBLK], BF16, tag="e", name="e_od")
                        st[f"e{pr}"] = e_od
                        nc.scalar.activation(e_od[:], ps_s[:], Exp, scale=SCALE)
                        Et = st["Et"]
                        if pr == 0:
                            nc.vector.tensor_add(
                                Et[:], e_od[:, 0:TBLK], e_od[:, ts(1, TBLK)]
                            )
                        else:
                            nc.vector.tensor_add(
                                Et[:], Et[:], e_od[:, 0:TBLK]
                            )
                            nc.vector.tensor_add(
                                Et[:], Et[:], e_od[:, ts(1, TBLK)]
                            )
                    s_units.append(s_od)

                def s_fin():
                    Et, eA, eB = st["Et"], st["eA"], st["eB"]
                    if qb > 0:
                        nc.vector.tensor_add(Et[:], Et[:], eA[:, 0:TBLK])
                    nc.vector.tensor_add(
                        Et[:, ds(P, 384)], Et[:, ds(P, 384)], eA[:, ds(TBLK, 384)]
                    )
                    nc.vector.tensor_add(
                        Et[:, ds(2 * P, 256)], Et[:, ds(2 * P, 256)], eB[:, 0:256]
                    )
                    nc.vector.tensor_add(
                        Et[:, ds(3 * P, P)], Et[:, ds(3 * P, P)], eB[:, ds(TBLK, P)]
                    )
                    E_bf = pE.tile([P, TBLK], BF16, tag="Eb", name="E_bf")
                    st["E_bf"] = E_bf
                    nc.vector.tensor_copy(E_bf[:], Et[:])
                s_units.append(s_fin)

                n_cmm = 4 * qb + 4
                cmm = [0]

                def c_mm(e_t, kt, e_col, w, q_off):
                    if cmm[0] == 0:
                        st["ps_ctx"] = pacc.tile(
                            [P, TBLK], F32, tag="acc", name="ps_ctx"
                        )
                    nc.tensor.matmul(
                        st["ps_ctx"][:, ds(q_off, w)],
                        lhsT=v_sb[:, b * (SEQ // P) + kt, ts(h, P)],
                        rhs=e_t[:, ds(e_col, w)],
                        start=(cmm[0] == 0),
                        stop=(cmm[0] == n_cmm - 1),
                    )
                    cmm[0] += 1

                def cA():
                    c_mm(st["eA"], 4 * qb + 0, 0, TBLK, 0)
                    c_mm(st["eA"], 4 * qb + 1, TBLK, 384, P)
                c_units.append(cA)

                def cB():
                    c_mm(st["eB"], 4 * qb + 2, 0, 256, 2 * P)
                    c_mm(st["eB"], 4 * qb + 3, TBLK, P, 3 * P)
                c_units.append(cB)

                for pr in range(2 * qb):
                    def c_od(pr=pr):
                        e_od = st[f"e{pr}"]
                        c_mm(e_od, 2 * pr, 0, TBLK, 0)
                        c_mm(e_od, 2 * pr + 1, TBLK, TBLK, 0)
                    c_units.append(c_od)

                def get_ep():
                    return (st["ps_ctx"], st["E_bf"], h, b, qb)

                st["get_ep"] = get_ep
                return s_units, c_units, st

            def emit_epilogue(ep, z_in_pps=False):
                ps_ctx, E_bf, h, b, qb = ep
                if z_in_pps:
                    ps_z = pps.tile([P, 2 * TBLK], F32, tag="ps", name="ps_z")
                    ps_z = ps_z[:, 0:TBLK]
                else:
                    ps_z = pacc.tile([P, TBLK], F32, tag="acc", name="ps_z")
                    ps_z = ps_z[:]
                nc.tensor.matmul(
                    ps_z, lhsT=ones_sb[:], rhs=E_bf[:],
                    start=True, stop=True,
                )
                lnz = pf.tile([P, TBLK], F32, tag="lnz", name="lnz")
                nc.scalar.activation(lnz[:], ps_z, Ln, scale=1.0)
                recip = pf.tile([P, TBLK], F32, tag="recip", name="recip")
                nc.scalar.activation(recip[:], lnz[:], Exp, scale=-1.0)
                ctxb = pf.tile([P, TBLK], BF16, tag="ctxb", name="ctxb")
                nc.vector.tensor_mul(ctxb[:], ps_ctx[:], recip[:])
                # scatter to the a2a staging buffer that owns these tokens
                if b == 0:
                    for j in range(2):
                        nc.sync.dma_start(
                            a2a_inA[2 * qb + j, h, :, :],
                            ctxb[:, ts(j, B0_SH)],
                        )
                elif qb < 3:
                    q0 = 512 * qb
                    for dst in range(N_CORES):
                        lo = max(q0, B1W * dst)
                        hi = min(q0 + 512, B1W * dst + B1W)
                        if lo < hi:
                            nc.sync.dma_start(
                                a2a_inB[0][dst, h, :, ds(lo - B1W * dst,
                                                         hi - lo)],
                                ctxb[:, ds(lo - q0, hi - lo)],
                            )
                else:
                    for dst in range(N_CORES):
                        nc.sync.dma_start(
                            a2a_inB[1][dst, h, :, :],
                            ctxb[:, ts(dst, B2W)],
                        )

            # ---- 8 sections: QKV(tb) + attention (h0,h1) at (b,qb)=tb ----
            pend = []          # epilogues carried into the next section
            wd_pre = []        # dense wd tiles prefetched during tb7
            for tb in range(NTB):
                b, qb = tb // 4, tb % 4
                if tb == NTB - 1:
                    # a2a#A completed long ago: its ctx loads go on sync
                    # here (no head-of-line blocking — the last x load was
                    # emitted above), then the first dense wd tiles.
                    emit_ctx_loads("A")
                    for j in range(4):
                        w_t = pwd.tile([P, KO, P], BF16, tag="wd")
                        nc.sync.dma_start(w_t[:], wd[j, :, :, :])
                        wd_pre.append(w_t)
                if tb == 0:
                    x_sb = x0_sb
                else:
                    x_sb = px.tile([P, KO, TBLK], BF16, tag="x")
                    for ko4 in range(4):
                        nc.sync.dma_start(
                            x_sb[:, ts(ko4, 4), :],
                            xT_r[:, ts(ko4, 4), ts(tb, TBLK)],
                        )

                def qk_pair(cp):
                    ps_qk = pps.tile([P, 2 * TBLK], F32, tag="ps", name="ps_qk")
                    for half in range(2):
                        ct = 2 * cp + half
                        for ko in range(KO):
                            nc.tensor.matmul(
                                ps_qk[:, ts(half, TBLK)],
                                lhsT=wqk_sb[:, ko, ts(ct, P)],
                                rhs=x_sb[:, ko, :],
                                start=(ko == 0),
                                stop=(ko == KO - 1),
                            )
                    for half in range(2):
                        ct = 2 * cp + half
                        nc.scalar.activation(
                            qk_sb[:, ct, ts(tb, TBLK)],
                            ps_qk[:, ts(half, TBLK)], Ident,
                            bias=bqk_sb[:, ct : ct + 1], scale=1.0,
                        )

                def v_group(vt):
                    ps_v = pacc.tile([P, 2 * P], F32, tag="acc", name="ps_v")
                    for ko in range(KO):
                        nc.tensor.matmul(
                            ps_v[:],
                            lhsT=x_sb[:, ko, ts(vt, P)],
                            rhs=wv_sb[:, ko, :],
                            start=(ko == 0),
                            stop=(ko == KO - 1),
                        )
                    nc.scalar.copy(v_sb[:, tb * (TBLK // P) + vt, :], ps_v[:])

                susA, cusA, stA = make_block(0, b, qb)
                susB, cusB, stB = make_block(1, b, qb)

                if tb == 0:
                    # cold start is DMA-bound: ko-major qk (4 concurrent
                    # PSUM accumulations) consumes only 256KB of fresh
                    # wqk+x per 1.05us of matmuls, which the HBM stream
                    # can actually sustain -- ct-major would idle the PE
                    # for ~8us waiting on the x chunks.
                    ps01 = pps.tile([P, 2 * TBLK], F32, tag="ps", name="ps01")
                    ps23 = pps.tile([P, 2 * TBLK], F32, tag="ps", name="ps23")
                    for ko in range(KO):
                        for pst, cp in ((ps01, 0), (ps23, 1)):
                            for half in range(2):
                                ct = 2 * cp + half
                                nc.tensor.matmul(
                                    pst[:, ts(half, TBLK)],
                                    lhsT=wqk_sb[:, ko, ts(ct, P)],
                                    rhs=x_sb[:, ko, :],
                                    start=(ko == 0),
                                    stop=(ko == KO - 1),
                                )
                    for pst, cp in ((ps01, 0), (ps23, 1)):
                        for half in range(2):
                            ct = 2 * cp + half
                            nc.scalar.activation(
                                qk_sb[:, ct, ts(tb, TBLK)],
                                pst[:, ts(half, TBLK)], Ident,
                                bias=bqk_sb[:, ct : ct + 1], scale=1.0,
                            )
                    v_group(0)
                    susA[0]()     # A.sA
                    v_group(1)
                    susA[1]()     # A.sAe
                    susA[2]()     # A.sB
                    v_group(2)
                    susA[3]()     # A.sBe
                    susB[0]()     # B.sA
                    v_group(3)
                    susB[1]()     # B.sAe
                    susB[2]()     # B.sB
                    susB[3]()     # B.sBe
                    susA[4]()     # A.s_fin
                    cusA[0]()     # A.cA
                    cusA[1]()     # A.cB
                    susB[4]()     # B.s_fin
                    cusB[0]()     # B.cA
                    cusB[1]()     # B.cB
                else:
                    # head: qk pairs + prev-section epilogues + diag blocks
                    qk_pair(0)
                    if pend:
                        emit_epilogue(pend[0])
                    v_group(0)
                    susA[0]()     # A.sA
                    if pend:
                        emit_epilogue(pend[1])
                        prev_tb = tb - 1
                        pend = []
                        if prev_tb == 3:
                            emit_a2a("A")
                        elif prev_tb == 6:
                            emit_a2a("0")
                    susA[1]()     # A.sAe
                    qk_pair(1)
                    susA[2]()     # A.sB
                    susA[3]()     # A.sBe
                    susB[0]()     # B.sA
                    susB[1]()     # B.sAe
                    susB[2]()     # B.sB
                    susB[3]()     # B.sBe
                    v_group(1)
                    if qb == 0:
                        susA[4]()     # fins
                        susB[4]()
                        v_group(2)
                        cusA[0]()
                        cusB[0]()
                        v_group(3)
                        cusA[1]()
                        cusB[1]()
                    else:
                        # tail: alternate od pairs between the two blocks,
                        # sprinkle remaining v-groups, ctx units lag 1 pair
                        ods = list(range(2 * qb))
                        ciA = ciB = 0
                        for k, pr in enumerate(ods):
                            susA[4 + pr]()
                            susB[4 + pr]()
                            if k == 0:
                                v_group(2)
                            elif k == 1:
                                v_group(3)
                            if k >= 1:
                                if ciA < len(cusA):
                                    cusA[ciA]()
                                    ciA += 1
                                if ciB < len(cusB):
                                    cusB[ciB]()
                                    ciB += 1
                        susA[4 + 2 * qb]()   # A.s_fin
                        susB[4 + 2 * qb]()   # B.s_fin
                        while ciA < len(cusA) or ciB < len(cusB):
                            if ciA < len(cusA):
                                cusA[ciA]()
                                ciA += 1
                            if ciB < len(cusB):
                                cusB[ciB]()
                                ciB += 1
                pend = [stA["get_ep"](), stB["get_ep"]()]

            # ---- dense projection, two disjoint-token passes ----
            # wd loads ride the scalar queue (x is done by then, out
            # writes are small); the collective-gated ctxT loads stay on
            # gpsimd so wd streaming never queues behind a collective
            # wait, and sync stays clear for the a2a_in writes.
            wd_q = wd_pre    # rolling prefetch queue, distance 4

            def wd_prefetch(src_ot, queue):
                w_t = pwd.tile([P, KO, P], BF16, tag="wd")
                queue.dma_start(w_t[:], wd[src_ot, :, :, :])
                wd_q.append(w_t)

            def dense_ot(ctx_sb, col0, ncol, ot, pre_ot=None, pre_q=None):
                wd_sb = wd_q.pop(0)
                if pre_ot is not None:
                    wd_prefetch(pre_ot, pre_q)
                ps_o = pacc.tile([P, ncol], F32, tag="acc", name="ps_o")
                for i in range(KO):
                    nc.tensor.matmul(
                        ps_o[:],
                        lhsT=wd_sb[:, i, :],
                        rhs=ctx_sb[:, i, :],
                        start=(i == 0),
                        stop=(i == KO - 1),
                    )
                out_sb = po.tile([P, ncol], F32, tag="osb")
                nc.scalar.activation(
                    out_sb[:], ps_o[:], Ident,
                    bias=bd_sb[:, ot : ot + 1], scale=1.0,
                )
                nc.scalar.dma_start(out[ts(ot, P), ds(col0, ncol)], out_sb[:])

            # ---- tail: last epilogues close a2a#B2 between the first
            # dense iterations, so the PE covers their ACT/DVE chains and
            # the collective fires ~10us earlier than a pure
            # epilogues-then-dense order would allow.
            # use-sequence: pass A ots 0..15, then pass B ots 0..15; the
            # prefetch for use u+4 is issued at use u (wd_pre seeds 0..3).
            # Pass-A tiles stream on sync (SP executes triggers instantly);
            # pass-B tiles on scalar so neither queue saturates.
            def pre_idx(u):
                n = u + 4
                if n < KO:
                    return n, nc.sync
                if n < 2 * KO:
                    return n - KO, nc.scalar
                return None, None

            # PE order dense0-2 first (their PSUM slots are free and their
            # operands long-loaded), then both epilogues back-to-back: by
            # then E_bf has drained from the DVE so the Z matmuls do not
            # stall the PE, while the softmax-scale chains and the a2a#B2
            # trigger still start within ~5us of the last attention work.
            dense_ot(ctxT_A, 0, B0_SH, 0, *pre_idx(0))
            dense_ot(ctxT_A, 0, B0_SH, 1, *pre_idx(1))
            dense_ot(ctxT_A, 0, B0_SH, 2, *pre_idx(2))
            emit_epilogue(pend[0], z_in_pps=True)
            emit_epilogue(pend[1], z_in_pps=True)
            pend = []
            emit_a2a("1")
            for ot in range(3, KO):
                dense_ot(ctxT_A, 0, B0_SH, ot, *pre_idx(ot))
            emit_ctx_loads("0")   # gated on a2a#B1, done by now
            emit_ctx_loads("1")   # gated on a2a#B2; nothing queues behind
            for ot in range(KO):
                dense_ot(ctxT_B, B0_SH, 2 * B1_SH, ot, *pre_idx(KO + ot))

    _patch_bass(nc)
    return nc


_cached_nc = None


def _get_nc():
    global _cached_nc
    if _cached_nc is None:
        _cached_nc = _build()
    return _cached_nc


# ----------------------------------------------------------------------------
# Host entry point
# ----------------------------------------------------------------------------
def kernel(x, mask, w_qkv, b_qkv, w_dense, b_dense):
    global _last_exec_time_ns, _last_res
    x = np.asarray(x, dtype=np.float32)
    w_qkv = np.asarray(w_qkv, dtype=np.float32)
    b_qkv = np.asarray(b_qkv, dtype=np.float32)
    w_dense = np.asarray(w_dense, dtype=np.float32)
    b_dense = np.asarray(b_dense, dtype=np.float32)

    bf16 = ml_dtypes.bfloat16
    # tokens batch-major: t = b*SEQ + s
    xT = np.ascontiguousarray(
        x.transpose(1, 0, 2).reshape(T, HIDDEN).T
    ).astype(bf16)
    # wd4[ot, p, ki, ocol] = w_dense.T[ki*128+p, ot*128+ocol]
    wd4 = np.ascontiguousarray(
        w_dense.T.reshape(KO, P, KO, P).transpose(2, 1, 0, 3)
    ).astype(bf16)

    # fold b_v into the dense bias: out = Wd@(probs@v) + (bd + Wd@bv)
    bv_full = np.empty(HIDDEN, dtype=np.float64)
    for hh in range(HEADS):
        bv_full[hh * HD:(hh + 1) * HD] = b_qkv[hh * 384 + 256: hh * 384 + 384]
    bd_eff = (
        b_dense.astype(np.float64) + w_dense.astype(np.float64) @ bv_full
    ).astype(np.float32)
    bd_host = np.ascontiguousarray(bd_eff.reshape(KO, P).T)

    in_maps = []
    for c in range(N_CORES):
        h0, h1 = 2 * c, 2 * c + 1
        qk_rows = np.concatenate(
            [
                np.arange(h0 * 384, h0 * 384 + 128),        # q_h0
                np.arange(h0 * 384 + 128, h0 * 384 + 256),  # k_h0
                np.arange(h1 * 384, h1 * 384 + 128),        # q_h1
                np.arange(h1 * 384 + 128, h1 * 384 + 256),  # k_h1
            ]
        )
        v_rows = np.concatenate(
            [
                np.arange(h0 * 384 + 256, h0 * 384 + 384),  # v_h0
                np.arange(h1 * 384 + 256, h1 * 384 + 384),  # v_h1
            ]
        )
        in_maps.append(
            {
                "xT": xT,
                "wqk": np.ascontiguousarray(w_qkv[qk_rows].T).astype(bf16),
                "wv": np.ascontiguousarray(w_qkv[v_rows].T).astype(bf16),
                "wd": wd4,
                "bqk": np.ascontiguousarray(b_qkv[qk_rows].reshape(4, P).T),
                "bd": bd_host,
            }
        )

    nc = _get_nc()
    trace = bool(int(os.environ.get("KERNEL_TRACE", "0")))
    if trace:
        trace = _install_ntff_hook()
    res = run_bass_kernel_spmd(
        nc, in_maps, core_ids=list(range(N_CORES)), trace=trace
    )
    _last_exec_time_ns = res.exec_time_ns
    _last_res = res

    # outs[c]["out"] is out^T [HIDDEN, 512]: cols 0:256 -> b0 tokens
    # [256c, 256c+256); cols 256:384 -> b1 tokens [128c, +128); cols
    # 384:512 -> b1 tokens [1024+128c, +128)   (b1 tokens offset 2048)
    full_T = np.empty((HIDDEN, T), dtype=np.float32)
    for c in range(N_CORES):
        o = res.results[c]["out"]
        full_T[:, 256 * c: 256 * c + 256] = o[:, 0:256]
        full_T[:, 2048 + 192 * c: 2048 + 192 * c + 192] = o[:, 256:448]
        full_T[:, 3584 + 64 * c: 3584 + 64 * c + 64] = o[:, 448:512]
    full = full_T.T  # [T, HIDDEN], batch-major tokens
    return np.ascontiguousarray(
        full.reshape(BATCH, SEQ, HIDDEN).transpose(1, 0, 2)
    ).astype(np.float32)


def last_exec_time_ns():
    return _last_exec_time_ns


# revision 41
# speedup vs baseline: 1.0138x; 1.0138x over previous
"""Distributed Trainium2 kernel for nn_Attention_65764539236808.

Multi-head causal self-attention layer (SEQ=2048, BATCH=2, HIDDEN=2048,
HEADS=16, HEAD_DIM=128) on 8 NeuronCores, tensor-parallel over heads
(2 heads/core).

v3 design (pipelined collectives; PE-roofline oriented):
  - 8 sections, one per 512-token block tb: QKV projection for tb
    interleaved with BOTH local heads' attention block (b, qb) =
    (tb//4, tb%4), which only needs K/V tiles produced by this and
    earlier sections.  So all of batch-0's ctx is finished at
    mid-kernel and batch-1's ctx finishes in two half-batch waves.
  - Three AllToAlls, each fully (or mostly) hidden:
      a2a#A  (b0 ctx, 1MB)   triggers inside section tb=4, covered by
                             sections 4-7 (~140us of PE work);
      a2a#B1 (b1 qb0-1, .5MB) triggers inside section tb=6;
      a2a#B2 (b1 qb2-3, .5MB) triggers right after section tb=7,
                             covered by dense pass A (~34us).
  - Token ownership per core c: 256 b0 tokens [256c,256c+256), plus
    128+128 b1 tokens [128c,+128) in each half of batch 1.  The dense
    projection runs as two disjoint-token passes (A: b0 cols 0:256,
    B: b1 cols 256:512), each applying the bias — no partial-sum
    buffer and no second all-reduce.
  Attention block internals (from v2):
  - scores^T layout [sk, sq]; diagonal 128x512 key-tiles only compute
    the un-masked column range (N = 512-128*kd) and run FIRST so their
    exp+mask chains finish before their ctx matmuls come up.
  - softmax denominator: e-tiles are accumulated into E_total on
    VectorE (f32); a single ones-matmul per block reduces over
    partitions in the epilogue ("a-mode" everywhere — the PE hybrid
    row-sum of v2 is gone, saving ~29k PE cycles, since every a2a
    trigger chain now has PE cover).
  - 1/Z via ScalarE exp(-ln(Z)); b_v folded into the dense bias on the
    host (sum(probs)==1).
  PSUM: one shared [128,1024] ring (2 bufs, 4 banks) carries qk-pairs
  and score pairs; a [128,512] ring (4 bufs) holds v-groups, ctx
  accumulators, Z and dense outputs.
"""

import math
import os
import sys
import types

import numpy as np
import ml_dtypes

import concourse.bass as bass
import concourse.mybir as mybir
import concourse.tile as tile
from concourse.bass import ts, ds
from concourse.bass_utils import run_bass_kernel_spmd

try:
    import orjson as _json_mod

    def _jloads(b):
        return _json_mod.loads(b)

    def _jdumps(o):
        return _json_mod.dumps(o)
except ImportError:  # pragma: no cover
    import json as _json_mod

    def _jloads(b):
        return _json_mod.loads(b)

    def _jdumps(o):
        return _json_mod.dumps(o).encode()

N_CORES = 8
SEQ, BATCH, HIDDEN, HEADS = 2048, 2, 2048, 16
HD = HIDDEN // HEADS          # 128
T = SEQ * BATCH               # 4096 tokens, batch-major: t = b*SEQ + s
P = 128
TBLK = 512                    # token block (free-dim tile)
NTB = T // TBLK               # 8
KO = HIDDEN // P              # 16 k-tiles over hidden
TOK_SHARD = T // N_CORES      # 512 tokens per core for the output
B0_SH = 256                   # per-core b0 token shard
B1_SH = 128                   # per-core b1 token shard per half
SCALE = 1.0 / math.sqrt(HD)

BF16 = mybir.dt.bfloat16
F32 = mybir.dt.float32

_last_exec_time_ns = None
_last_res = None


# ----------------------------------------------------------------------------
# Workaround: this walrus build accepts only ONE sync-wait per instruction.
# Hoist extra on_wait entries onto single-wait EventSemaphore instructions
# inserted just before the owner (same engine => same program order, so the
# semantics are identical).
# ----------------------------------------------------------------------------
def _split_multiwait(bir: dict) -> dict:
    ctr = 0
    for fn in bir.get("functions", []):
        for blk in fn.get("blocks", []):
            insts = blk.get("instructions")
            if not insts:
                continue
            new_insts = []
            changed = False
            for inst in insts:
                si = inst.get("sync_info")
                ow = (si or {}).get("on_wait") or []
                if len(ow) > 1:
                    changed = True
                    for w in ow[:-1]:
                        ctr += 1
                        new_insts.append(
                            {
                                "debug": inst.get("debug", 0),
                                "engine": inst["engine"],
                                "ins": [],
                                "name": f"{inst['name']}-mw{ctr}",
                                "opcode": "EventSemaphore",
                                "outs": [],
                                "sync_info": {"on_update": [], "on_wait": [w]},
                            }
                        )
                    si["on_wait"] = [ow[-1]]
                new_insts.append(inst)
            if changed:
                blk["instructions"] = new_insts
    return bir


def _patch_bass(nc):
    if getattr(nc, "_waitfix_patched", False):
        return nc
    orig = nc.to_json_bytes

    def patched():
        return _jdumps(_split_multiwait(_jloads(orig())))

    nc.to_json_bytes = patched
    nc._waitfix_patched = True
    return nc


def _install_ntff_hook():
    """Recreate antenv.axon_hooks if the image lacks it (needed for trace=True)."""
    try:
        from antenv.axon_hooks import get_axon_ntff_profile_hook  # noqa: F401
        return True
    except ImportError:
        pass
    try:
        from trn_agent_boot.trn_boot import _ntff_profile_via_ctypes

        hook = _ntff_profile_via_ctypes("/opt/axon/libaxon_pjrt.so")
        if hook is None:
            return False
        mod = types.ModuleType("antenv.axon_hooks")
        mod._hook = hook
        mod.get_axon_ntff_profile_hook = lambda: mod._hook
        mod.set_axon_ntff_profile_hook = lambda h: setattr(mod, "_hook", h)
        sys.modules["antenv.axon_hooks"] = mod
        import antenv

        antenv.axon_hooks = mod
        return True
    except Exception:
        return False


# ----------------------------------------------------------------------------
# Device graph (SPMD: same graph on all 8 cores)
# ----------------------------------------------------------------------------
def _build():
    nc = bass.Bass()

    xT = nc.declare_dram_parameter("xT", [HIDDEN, T], BF16, isOutput=False)
    wqk = nc.declare_dram_parameter("wqk", [HIDDEN, 4 * P], BF16, isOutput=False)
    wv = nc.declare_dram_parameter("wv", [HIDDEN, 2 * P], BF16, isOutput=False)
    # wd pre-tiled on the host as [ot, p, ki, ocol] so each per-ot load is
    # one contiguous 4KB-per-partition DMA
    wd = nc.declare_dram_parameter("wd", [KO, P, KO, P], BF16, isOutput=False)
    bqk = nc.declare_dram_parameter("bqk", [P, 4], F32, isOutput=False)
    bd = nc.declare_dram_parameter("bd", [P, KO], F32, isOutput=False)  # + Wd@bv
    out = nc.declare_dram_parameter("out", [HIDDEN, TOK_SHARD], F32, isOutput=True)

    xT_r = xT.rearrange("(ko p) t -> p ko t", p=P)
    wqk_r = wqk.rearrange("(ko p) c -> p ko c", p=P)
    wv_r = wv.rearrange("(ko p) c -> p ko c", p=P)

    Exp = mybir.ActivationFunctionType.Exp
    Ln = mybir.ActivationFunctionType.Ln
    Ident = mybir.ActivationFunctionType.Identity

    with tile.TileContext(nc) as tc:
        with (
            tc.tile_pool(name="const", bufs=1) as pc,
            tc.tile_pool(name="xs", bufs=2) as px,
            tc.tile_pool(name="es", bufs=14) as pe,
            tc.tile_pool(name="esum", bufs=3) as pE,
            tc.tile_pool(name="fs", bufs=3) as pf,
            tc.tile_pool(name="wds", bufs=6) as pwd,
            tc.tile_pool(name="outs", bufs=3) as po,
            tc.tile_pool(name="ps_ring", bufs=2, space="PSUM") as pps,
            tc.tile_pool(name="ps_acc", bufs=4, space="PSUM") as pacc,
            tc.tile_pool(name="dram", bufs=1, space="DRAM") as pdram,
        ):
            # ---- constants ----
            # preamble rides three queues in parallel: wqk on sync, x0 on
            # scalar, wv on gpsimd — the tb0 ko-major qk loop consumes
            # wqk[ko]+x0[ko] at ~250GB/s aggregate, which one queue alone
            # cannot sustain.
            wqk_sb = pc.tile([P, KO, 4 * P], BF16)
            x0_sb = px.tile([P, KO, TBLK], BF16, tag="x")
            for lo, n in [(0, 1), (1, 1), (2, 1), (3, 1), (4, 2), (6, 2),
                          (8, 4), (12, 4)]:
                nc.sync.dma_start(
                    wqk_sb[:, ds(lo, n), :], wqk_r[:, ds(lo, n), :]
                )
                nc.sync.dma_start(
                    x0_sb[:, ds(lo, n), :], xT_r[:, ds(lo, n), ts(0, TBLK)]
                )
            wv_sb = pc.tile([P, KO, 2 * P], BF16)
            for ko4 in range(4):
                nc.gpsimd.dma_start(
                    wv_sb[:, ts(ko4, 4), :], wv_r[:, ts(ko4, 4), :]
                )
            bqk_sb = pc.tile([P, 4], F32)
            nc.sync.dma_start(bqk_sb[:], bqk[:])
            bd_sb = pc.tile([P, KO], F32)
            nc.sync.dma_start(bd_sb[:], bd[:])

            ones_sb = pc.tile([P, P], BF16)
            nc.vector.memset(ones_sb[:], 1.0)
            # triangular mask [sk, sq_local]: keep where sq >= sk
            tri_sb = pc.tile([P, P], BF16)
            nc.vector.memset(tri_sb[:], 1.0)
            nc.gpsimd.affine_select(
                out=tri_sb[:],
                in_=tri_sb[:],
                compare_op=mybir.AluOpType.is_ge,
                fill=0.0,
                base=0,
                pattern=[[1, P]],
                channel_multiplier=-1,
            )

            qk_sb = pc.tile([P, 4, T], BF16)   # [d, (q_h0,k_h0,q_h1,k_h1), tok]
            v_sb = pc.tile([P, T // P, 2 * P], BF16)  # [tok_in_tile, tile, (v0,v1)]

            # a2a staging: A carries b0 ctx (dst shard 256 tokens),
            # B1/B2 carry one half of b1 each (dst shard 128 tokens).
            a2a_inA = pdram.tile([N_CORES, 2, P, B0_SH], BF16,
                                 name="a2a_inA", tag="a2a_inA")
            a2a_outA = pdram.tile([N_CORES, 2, P, B0_SH], BF16,
                                  name="a2a_outA", tag="a2a_outA")
            # b1 ships in three waves so every dense input is on-core well
            # before its pass starts, whatever the fabric latency: B1 =
            # query-blocks 0-1 (128-token shards, trigger 2 sections before
            # the end), B2 = block 2 (64, trigger 1 section before), B3 =
            # block 3 (64, trigger at the tail under ~50us of dense cover).
            BW = [128, 64, 64]
            a2a_inB = [
                pdram.tile([N_CORES, 2, P, BW[k]], BF16,
                           name=f"a2a_inB{k}", tag=f"a2a_inB{k}")
                for k in range(3)
            ]
            a2a_outB = [
                pdram.tile([N_CORES, 2, P, BW[k]], BF16,
                           name=f"a2a_outB{k}", tag=f"a2a_outB{k}")
                for k in range(3)
            ]
            # [d, global head, tok] gathered ctx for this core's shard:
            # cols 0:256 b0, 256:384 b1-qb01, 384:448 qb2, 448:512 qb3
            ctxT = pc.tile([P, HEADS, TOK_SHARD], BF16, name="ctxT")
            BOFF = [B0_SH, B0_SH + 128, B0_SH + 192]

            # collective triggers ride gpsimd (and ONLY the triggers: the
            # gpsimd SWDGE costs ~3.3us per dma op, serially, so any loads
            # placed there both run late and block later triggers).  The
            # gathered-ctx loads ride the sync HWDGE queue instead, emitted
            # at a point where their gating collective is already complete
            # so they never head-of-line-block the x stream.
            def emit_a2a(which):
                if which == "A":
                    ins, outs = a2a_inA, a2a_outA
                else:
                    ins, outs = a2a_inB[int(which)], a2a_outB[int(which)]
                nc.gpsimd.collective_compute(
                    "AllToAll",
                    mybir.AluOpType.bypass,
                    replica_groups=[list(range(N_CORES))],
                    ins=[ins[:].opt()],
                    outs=[outs[:].opt()],
                )

            def emit_ctx_loads(which):
                if which == "A":
                    src, c0, w = a2a_outA, 0, B0_SH
                else:
                    k = int(which)
                    src, c0, w = a2a_outB[k], BOFF[k], BW[k]
                for i in range(N_CORES):
                    for h in range(2):
                        nc.sync.dma_start(
                            ctxT[:, 2 * i + h, ds(c0, w)],
                            src[i, h, :, :],
                        )

            # ------------------------------------------------------------
            # attention block emission.  Returns (s_units, c_units, state):
            # score units (diag packs first, then off-diag pairs, then the
            # E finisher) and ctx-matmul units.  Row sums accumulate into
            # E on the DVE ("a-mode"); the Z ones-matmul runs in the
            # epilogue.
            # ------------------------------------------------------------
            def make_block(h, b, qb):
                base = b * SEQ
                q0 = base + qb * TBLK
                st = {}

                def k_ap(kt):
                    return qk_sb[:, 2 * h + 1, ds(base + kt * P, P)]

                s_units = []
                c_units = []

                def sA():
                    st["Et"] = pE.tile([P, TBLK], F32, tag="Ef", name="Et")
                    psA = pps.tile([P, 2 * TBLK], F32, tag="ps", name="psA")
                    st["psA"] = psA
                    nc.tensor.matmul(
                        psA[:, 0:TBLK], lhsT=k_ap(4 * qb + 0),
                        rhs=qk_sb[:, 2 * h, ds(q0, TBLK)], start=True, stop=True,
                    )
                    nc.tensor.matmul(
                        psA[:, ds(TBLK, 384)], lhsT=k_ap(4 * qb + 1),
                        rhs=qk_sb[:, 2 * h, ds(q0 + P, 384)], start=True, stop=True,
                    )
                s_units.append(sA)

                def sAe():
                    eA = pe.tile([P, 2 * TBLK], BF16, tag="e", name="eA")
                    st["eA"] = eA
                    nc.scalar.activation(
                        eA[:, 0:TBLK + 384], st["psA"][:, 0:TBLK + 384],
                        Exp, scale=SCALE,
                    )
                    nc.vector.tensor_mul(eA[:, 0:P], eA[:, 0:P], tri_sb[:])
                    nc.vector.tensor_mul(
                        eA[:, ds(TBLK, P)], eA[:, ds(TBLK, P)], tri_sb[:]
                    )
                    if qb == 0:
                        nc.vector.tensor_copy(st["Et"][:], eA[:, 0:TBLK])
                s_units.append(sAe)

                def sB():
                    psB = pps.tile([P, 2 * TBLK], F32, tag="ps", name="psB")
                    st["psB"] = psB
                    nc.tensor.matmul(
                        psB[:, 0:256], lhsT=k_ap(4 * qb + 2),
                        rhs=qk_sb[:, 2 * h, ds(q0 + 2 * P, 256)],
                        start=True, stop=True,
                    )
                    nc.tensor.matmul(
                        psB[:, ds(TBLK, P)], lhsT=k_ap(4 * qb + 3),
                        rhs=qk_sb[:, 2 * h, ds(q0 + 3 * P, P)],
                        start=True, stop=True,
                    )
                s_units.append(sB)

                def sBe():
                    eB = pe.tile([P, 2 * TBLK], BF16, tag="e", name="eB")
                    st["eB"] = eB
                    nc.scalar.activation(
                        eB[:, 0:TBLK + P], st["psB"][:, 0:TBLK + P],
                        Exp, scale=SCALE,
                    )
                    nc.vector.tensor_mul(eB[:, 0:P], eB[:, 0:P], tri_sb[:])
                    nc.vector.tensor_mul(
                        eB[:, ds(TBLK, P)], eB[:, ds(TBLK, P)], tri_sb[:]
                    )
                s_units.append(sBe)

                for pr in range(2 * qb):
                    def s_od(pr=pr):
                        ps_s = pps.tile([P, 2 * TBLK], F32, tag="ps", name="ps_s")
                        for half in range(2):
                            kt = 2 * pr + half
                            nc.tensor.matmul(
                                ps_s[:, ts(half, TBLK)],
                                lhsT=k_ap(kt),
                                rhs=qk_sb[:, 2 * h, ds(q0, TBLK)],
                                start=True, stop=True,
                            )
                        e_od = pe.tile([P, 2 * TBLK], BF16, tag="e", name="e_od")
                        st[f"e{pr}"] = e_od
                        nc.scalar.activation(e_od[:], ps_s[:], Exp, scale=SCALE)
                        Et = st["Et"]
                        if pr == 0:
                            nc.vector.tensor_add(
                                Et[:], e_od[:, 0:TBLK], e_od[:, ts(1, TBLK)]
                            )
                        else:
                            nc.vector.tensor_add(
                                Et[:], Et[:], e_od[:, 0:TBLK]
                            )
                            nc.vector.tensor_add(
                                Et[:], Et[:], e_od[:, ts(1, TBLK)]
                            )
                    s_units.append(s_od)

                def s_fin():
                    Et, eA, eB = st["Et"], st["eA"], st["eB"]
                    if qb > 0:
                        nc.vector.tensor_add(Et[:], Et[:], eA[:, 0:TBLK])
                    nc.vector.tensor_add(
                        Et[:, ds(P, 384)], Et[:, ds(P, 384)], eA[:, ds(TBLK, 384)]
                    )
                    nc.vector.tensor_add(
                        Et[:, ds(2 * P, 256)], Et[:, ds(2 * P, 256)], eB[:, 0:256]
                    )
                    nc.vector.tensor_add(
                        Et[:, ds(3 * P, P)], Et[:, ds(3 * P, P)], eB[:, ds(TBLK, P)]
                    )
                    E_bf = pE.tile([P, TBLK], BF16, tag="Eb", name="E_bf")
                    st["E_bf"] = E_bf
                    nc.vector.tensor_copy(E_bf[:], Et[:])
                s_units.append(s_fin)

                n_cmm = 4 * qb + 4
                cmm = [0]

                def c_mm(e_t, kt, e_col, w, q_off):
                    if cmm[0] == 0:
                        st["ps_ctx"] = pacc.tile(
                            [P, TBLK], F32, tag="acc", name="ps_ctx"
                        )
                    nc.tensor.matmul(
                        st["ps_ctx"][:, ds(q_off, w)],
                        lhsT=v_sb[:, b * (SEQ // P) + kt, ts(h, P)],
                        rhs=e_t[:, ds(e_col, w)],
                        start=(cmm[0] == 0),
                        stop=(cmm[0] == n_cmm - 1),
                    )
                    cmm[0] += 1

                def cA():
                    c_mm(st["eA"], 4 * qb + 0, 0, TBLK, 0)
                    c_mm(st["eA"], 4 * qb + 1, TBLK, 384, P)
                c_units.append(cA)

                def cB():
                    c_mm(st["eB"], 4 * qb + 2, 0, 256, 2 * P)
                    c_mm(st["eB"], 4 * qb + 3, TBLK, P, 3 * P)
                c_units.append(cB)

                for pr in range(2 * qb):
                    def c_od(pr=pr):
                        e_od = st[f"e{pr}"]
                        c_mm(e_od, 2 * pr, 0, TBLK, 0)
                        c_mm(e_od, 2 * pr + 1, TBLK, TBLK, 0)
                    c_units.append(c_od)

                def get_ep():
                    return (st["ps_ctx"], st["E_bf"], h, b, qb)

                st["get_ep"] = get_ep
                return s_units, c_units, st

            def emit_epilogue(ep, z_in_pps=False):
                ps_ctx, E_bf, h, b, qb = ep
                if z_in_pps:
                    ps_z = pps.tile([P, 2 * TBLK], F32, tag="ps", name="ps_z")
                    ps_z = ps_z[:, 0:TBLK]
                else:
                    ps_z = pacc.tile([P, TBLK], F32, tag="acc", name="ps_z")
                    ps_z = ps_z[:]
                nc.tensor.matmul(
                    ps_z, lhsT=ones_sb[:], rhs=E_bf[:],
                    start=True, stop=True,
                )
                lnz = pf.tile([P, TBLK], F32, tag="lnz", name="lnz")
                nc.scalar.activation(lnz[:], ps_z, Ln, scale=1.0)
                recip = pf.tile([P, TBLK], F32, tag="recip", name="recip")
                nc.scalar.activation(recip[:], lnz[:], Exp, scale=-1.0)
                ctxb = pf.tile([P, TBLK], BF16, tag="ctxb", name="ctxb")
                nc.vector.tensor_mul(ctxb[:], ps_ctx[:], recip[:])
                # scatter to the a2a staging buffer that owns these tokens
                if b == 0:
                    for j in range(2):
                        nc.sync.dma_start(
                            a2a_inA[2 * qb + j, h, :, :],
                            ctxb[:, ts(j, B0_SH)],
                        )
                elif qb < 2:
                    for j in range(4):
                        nc.sync.dma_start(
                            a2a_inB[0][4 * qb + j, h, :, :],
                            ctxb[:, ts(j, BW[0])],
                        )
                else:
                    for dst in range(N_CORES):
                        nc.sync.dma_start(
                            a2a_inB[qb - 1][dst, h, :, :],
                            ctxb[:, ts(dst, 64)],
                        )

            # ---- 8 sections: QKV(tb) + attention (h0,h1) at (b,qb)=tb ----
            pend = []          # epilogues carried into the next section
            wd_pre = []        # dense wd tiles prefetched during tb7
            for tb in range(NTB):
                b, qb = tb // 4, tb % 4
                if tb == NTB - 1:
                    # a2a#A completed long ago: its ctx loads go on sync
                    # here (no head-of-line blocking — the last x load was
                    # emitted above), then the first dense wd tiles, then
                    # the B1 ctx loads (gated on a2a#B1, done by ~2
                    # sections later at the latest).
                    emit_ctx_loads("A")
                    for j in range(4):
                        w_t = pwd.tile([P, KO, P], BF16, tag="wd")
                        nc.sync.dma_start(w_t[:], wd[j, :, :, :])
                        wd_pre.append(w_t)
                    emit_ctx_loads("0")
                if tb == 0:
                    x_sb = x0_sb
                else:
                    x_sb = px.tile([P, KO, TBLK], BF16, tag="x")
                    for ko4 in range(4):
                        nc.sync.dma_start(
                            x_sb[:, ts(ko4, 4), :],
                            xT_r[:, ts(ko4, 4), ts(tb, TBLK)],
                        )

                def qk_pair(cp):
                    ps_qk = pps.tile([P, 2 * TBLK], F32, tag="ps", name="ps_qk")
                    for half in range(2):
                        ct = 2 * cp + half
                        for ko in range(KO):
                            nc.tensor.matmul(
                                ps_qk[:, ts(half, TBLK)],
                                lhsT=wqk_sb[:, ko, ts(ct, P)],
                                rhs=x_sb[:, ko, :],
                                start=(ko == 0),
                                stop=(ko == KO - 1),
                            )
                    for half in range(2):
                        ct = 2 * cp + half
                        nc.scalar.activation(
                            qk_sb[:, ct, ts(tb, TBLK)],
                            ps_qk[:, ts(half, TBLK)], Ident,
                            bias=bqk_sb[:, ct : ct + 1], scale=1.0,
                        )

                def v_group(vt):
                    ps_v = pacc.tile([P, 2 * P], F32, tag="acc", name="ps_v")
                    for ko in range(KO):
                        nc.tensor.matmul(
                            ps_v[:],
                            lhsT=x_sb[:, ko, ts(vt, P)],
                            rhs=wv_sb[:, ko, :],
                            start=(ko == 0),
                            stop=(ko == KO - 1),
                        )
                    nc.scalar.copy(v_sb[:, tb * (TBLK // P) + vt, :], ps_v[:])

                susA, cusA, stA = make_block(0, b, qb)
                susB, cusB, stB = make_block(1, b, qb)

                if tb == 0:
                    # cold start is DMA-bound: ko-major qk (4 concurrent
                    # PSUM accumulations) consumes only 256KB of fresh
                    # wqk+x per 1.05us of matmuls, which the HBM stream
                    # can actually sustain -- ct-major would idle the PE
                    # for ~8us waiting on the x chunks.
                    ps01 = pps.tile([P, 2 * TBLK], F32, tag="ps", name="ps01")
                    ps23 = pps.tile([P, 2 * TBLK], F32, tag="ps", name="ps23")
                    for ko in range(KO):
                        for pst, cp in ((ps01, 0), (ps23, 1)):
                            for half in range(2):
                                ct = 2 * cp + half
                                nc.tensor.matmul(
                                    pst[:, ts(half, TBLK)],
                                    lhsT=wqk_sb[:, ko, ts(ct, P)],
                                    rhs=x_sb[:, ko, :],
                                    start=(ko == 0),
                                    stop=(ko == KO - 1),
                                )
                    for pst, cp in ((ps01, 0), (ps23, 1)):
                        for half in range(2):
                            ct = 2 * cp + half
                            nc.scalar.activation(
                                qk_sb[:, ct, ts(tb, TBLK)],
                                pst[:, ts(half, TBLK)], Ident,
                                bias=bqk_sb[:, ct : ct + 1], scale=1.0,
                            )
                    v_group(0)
                    susA[0]()     # A.sA
                    v_group(1)
                    susA[1]()     # A.sAe
                    susA[2]()     # A.sB
                    v_group(2)
                    susA[3]()     # A.sBe
                    susB[0]()     # B.sA
                    v_group(3)
                    susB[1]()     # B.sAe
                    susB[2]()     # B.sB
                    susB[3]()     # B.sBe
                    susA[4]()     # A.s_fin
                    cusA[0]()     # A.cA
                    cusA[1]()     # A.cB
                    susB[4]()     # B.s_fin
                    cusB[0]()     # B.cA
                    cusB[1]()     # B.cB
                else:
                    # head: qk pairs + prev-section epilogues + diag blocks
                    qk_pair(0)
                    if pend:
                        emit_epilogue(pend[0])
                    v_group(0)
                    susA[0]()     # A.sA
                    if pend:
                        emit_epilogue(pend[1])
                        prev_tb = tb - 1
                        pend = []
                        if prev_tb == 3:
                            emit_a2a("A")
                        elif prev_tb == 5:
                            emit_a2a("0")
                        elif prev_tb == 6:
                            emit_a2a("1")
                    susA[1]()     # A.sAe
                    qk_pair(1)
                    susA[2]()     # A.sB
                    susA[3]()     # A.sBe
                    susB[0]()     # B.sA
                    susB[1]()     # B.sAe
                    susB[2]()     # B.sB
                    susB[3]()     # B.sBe
                    v_group(1)
                    if qb == 0:
                        susA[4]()     # fins
                        susB[4]()
                        v_group(2)
                        cusA[0]()
                        cusB[0]()
                        v_group(3)
                        cusA[1]()
                        cusB[1]()
                    else:
                        # tail: alternate od pairs between the two blocks,
                        # sprinkle remaining v-groups, ctx units lag 1 pair
                        ods = list(range(2 * qb))
                        ciA = ciB = 0
                        for k, pr in enumerate(ods):
                            susA[4 + pr]()
                            susB[4 + pr]()
                            if k == 0:
                                v_group(2)
                            elif k == 1:
                                v_group(3)
                            if k >= 1:
                                if ciA < len(cusA):
                                    cusA[ciA]()
                                    ciA += 1
                                if ciB < len(cusB):
                                    cusB[ciB]()
                                    ciB += 1
                        susA[4 + 2 * qb]()   # A.s_fin
                        susB[4 + 2 * qb]()   # B.s_fin
                        while ciA < len(cusA) or ciB < len(cusB):
                            if ciA < len(cusA):
                                cusA[ciA]()
                                ciA += 1
                            if ciB < len(cusB):
                                cusB[ciB]()
                                ciB += 1
                pend = [stA["get_ep"](), stB["get_ep"]()]

            # ---- dense projection, two disjoint-token passes ----
            # wd loads ride the scalar queue (x is done by then, out
            # writes are small); the collective-gated ctxT loads stay on
            # gpsimd so wd streaming never queues behind a collective
            # wait, and sync stays clear for the a2a_in writes.
            wd_q = wd_pre    # rolling prefetch queue, distance 4

            def wd_prefetch(src_ot):
                w_t = pwd.tile([P, KO, P], BF16, tag="wd")
                nc.scalar.dma_start(w_t[:], wd[src_ot, :, :, :])
                wd_q.append(w_t)

            def dense_ot(col0, ncol, ot, pre_ot=None):
                wd_sb = wd_q.pop(0)
                if pre_ot is not None:
                    wd_prefetch(pre_ot)
                ps_o = pacc.tile([P, ncol], F32, tag="acc", name="ps_o")
                for i in range(KO):
                    nc.tensor.matmul(
                        ps_o[:],
                        lhsT=wd_sb[:, i, :],
                        rhs=ctxT[:, i, ds(col0, ncol)],
                        start=(i == 0),
                        stop=(i == KO - 1),
                    )
                out_sb = po.tile([P, ncol], F32, tag="osb")
                nc.scalar.activation(
                    out_sb[:], ps_o[:], Ident,
                    bias=bd_sb[:, ot : ot + 1], scale=1.0,
                )
                nc.scalar.dma_start(out[ts(ot, P), ds(col0, ncol)], out_sb[:])

            # ---- tail: last epilogues close a2a#B2 between the first
            # dense iterations, so the PE covers their ACT/DVE chains and
            # the collective fires ~10us earlier than a pure
            # epilogues-then-dense order would allow.
            # use-sequence: pass A ots 0..15, then pass B ots 0..15; the
            # prefetch for use u+4 is issued at use u (wd_pre seeds 0..3)
            def pre_idx(u):
                n = u + 4
                if n < KO:
                    return n
                if n < 2 * KO:
                    return n - KO
                return None

            # PE order: dense0 first (its PSUM slot is free and operands
            # long-loaded, so it bridges the ACT/DVE epilogue tail), then
            # both epilogues back-to-back: E_bf has drained from the DVE
            # so the Z matmuls barely stall the PE, and the a2a#B3 chain
            # (one bias ahead of ln on ACT) fires ~6us after the last
            # attention work, under ~34us of pass-A cover.
            dense_ot(0, B0_SH, 0, pre_idx(0))
            emit_epilogue(pend[0], z_in_pps=True)
            emit_epilogue(pend[1], z_in_pps=True)
            pend = []
            emit_a2a("2")
            for ot in range(1, KO):
                dense_ot(0, B0_SH, ot, pre_idx(ot))
            emit_ctx_loads("1")   # gated on a2a#B2, done by now
            emit_ctx_loads("2")   # gated on a2a#B3; nothing queues behind
            for ot in range(KO):
                dense_ot(B0_SH, B0_SH, ot, pre_idx(KO + ot))

    _patch_bass(nc)
    return nc


_cached_nc = None


def _get_nc():
    global _cached_nc
    if _cached_nc is None:
        _cached_nc = _build()
    return _cached_nc


# ----------------------------------------------------------------------------
# Host entry point
# ----------------------------------------------------------------------------
def kernel(x, mask, w_qkv, b_qkv, w_dense, b_dense):
    global _last_exec_time_ns, _last_res
    x = np.asarray(x, dtype=np.float32)
    w_qkv = np.asarray(w_qkv, dtype=np.float32)
    b_qkv = np.asarray(b_qkv, dtype=np.float32)
    w_dense = np.asarray(w_dense, dtype=np.float32)
    b_dense = np.asarray(b_dense, dtype=np.float32)

    bf16 = ml_dtypes.bfloat16
    # tokens batch-major: t = b*SEQ + s
    xT = np.ascontiguousarray(
        x.transpose(1, 0, 2).reshape(T, HIDDEN).T
    ).astype(bf16)
    # wd4[ot, p, ki, ocol] = w_dense.T[ki*128+p, ot*128+ocol]
    wd4 = np.ascontiguousarray(
        w_dense.T.reshape(KO, P, KO, P).transpose(2, 1, 0, 3)
    ).astype(bf16)

    # fold b_v into the dense bias: out = Wd@(probs@v) + (bd + Wd@bv)
    bv_full = np.empty(HIDDEN, dtype=np.float64)
    for hh in range(HEADS):
        bv_full[hh * HD:(hh + 1) * HD] = b_qkv[hh * 384 + 256: hh * 384 + 384]
    bd_eff = (
        b_dense.astype(np.float64) + w_dense.astype(np.float64) @ bv_full
    ).astype(np.float32)
    bd_host = np.ascontiguousarray(bd_eff.reshape(KO, P).T)

    in_maps = []
    for c in range(N_CORES):
        h0, h1 = 2 * c, 2 * c + 1
        qk_rows = np.concatenate(
            [
                np.arange(h0 * 384, h0 * 384 + 128),        # q_h0
                np.arange(h0 * 384 + 128, h0 * 384 + 256),  # k_h0
                np.arange(h1 * 384, h1 * 384 + 128),        # q_h1
                np.arange(h1 * 384 + 128, h1 * 384 + 256),  # k_h1
            ]
        )
        v_rows = np.concatenate(
            [
                np.arange(h0 * 384 + 256, h0 * 384 + 384),  # v_h0
                np.arange(h1 * 384 + 256, h1 * 384 + 384),  # v_h1
            ]
        )
        in_maps.append(
            {
                "xT": xT,
                "wqk": np.ascontiguousarray(w_qkv[qk_rows].T).astype(bf16),
                "wv": np.ascontiguousarray(w_qkv[v_rows].T).astype(bf16),
                "wd": wd4,
                "bqk": np.ascontiguousarray(b_qkv[qk_rows].reshape(4, P).T),
                "bd": bd_host,
            }
        )

    nc = _get_nc()
    trace = bool(int(os.environ.get("KERNEL_TRACE", "0")))
    if trace:
        trace = _install_ntff_hook()
    res = run_bass_kernel_spmd(
        nc, in_maps, core_ids=list(range(N_CORES)), trace=trace
    )
    _last_exec_time_ns = res.exec_time_ns
    _last_res = res

    # outs[c]["out"] is out^T [HIDDEN, 512]: cols 0:256 -> b0 tokens
    # [256c, 256c+256); cols 256:384 -> b1 tokens [128c, +128); cols
    # 384:512 -> b1 tokens [1024+128c, +128)   (b1 tokens offset 2048)
    full_T = np.empty((HIDDEN, T), dtype=np.float32)
    for c in range(N_CORES):
        o = res.results[c]["out"]
        full_T[:, 256 * c: 256 * c + 256] = o[:, 0:256]
        full_T[:, 2048 + 128 * c: 2048 + 128 * c + 128] = o[:, 256:384]
        full_T[:, 3072 + 64 * c: 3072 + 64 * c + 64] = o[:, 384:448]
        full_T[:, 3584 + 64 * c: 3584 + 64 * c + 64] = o[:, 448:512]
    full = full_T.T  # [T, HIDDEN], batch-major tokens
    return np.ascontiguousarray(
        full.reshape(BATCH, SEQ, HIDDEN).transpose(1, 0, 2)
    ).astype(np.float32)


def last_exec_time_ns():
    return _last_exec_time_ns


# revision 50
# speedup vs baseline: 1.0164x; 1.0025x over previous
"""Distributed Trainium2 kernel for nn_Attention_65764539236808.

Multi-head causal self-attention layer (SEQ=2048, BATCH=2, HIDDEN=2048,
HEADS=16, HEAD_DIM=128) on 8 NeuronCores, tensor-parallel over heads
(2 heads/core).

v3 design (pipelined collectives; PE-roofline oriented):
  - 8 sections, one per 512-token block tb: QKV projection for tb
    interleaved with BOTH local heads' attention block (b, qb) =
    (tb//4, tb%4), which only needs K/V tiles produced by this and
    earlier sections.  So all of batch-0's ctx is finished at
    mid-kernel and batch-1's ctx finishes in two half-batch waves.
  - Three AllToAlls, each fully (or mostly) hidden:
      a2a#A  (b0 ctx, 1MB)   triggers inside section tb=4, covered by
                             sections 4-7 (~140us of PE work);
      a2a#B1 (b1 qb0-1, .5MB) triggers inside section tb=6;
      a2a#B2 (b1 qb2-3, .5MB) triggers right after section tb=7,
                             covered by dense pass A (~34us).
  - Token ownership per core c: 256 b0 tokens [256c,256c+256), plus
    128+128 b1 tokens [128c,+128) in each half of batch 1.  The dense
    projection runs as two disjoint-token passes (A: b0 cols 0:256,
    B: b1 cols 256:512), each applying the bias — no partial-sum
    buffer and no second all-reduce.
  Attention block internals (from v2):
  - scores^T layout [sk, sq]; diagonal 128x512 key-tiles only compute
    the un-masked column range (N = 512-128*kd) and run FIRST so their
    exp+mask chains finish before their ctx matmuls come up.
  - softmax denominator: e-tiles are accumulated into E_total on
    VectorE (f32); a single ones-matmul per block reduces over
    partitions in the epilogue ("a-mode" everywhere — the PE hybrid
    row-sum of v2 is gone, saving ~29k PE cycles, since every a2a
    trigger chain now has PE cover).
  - 1/Z via ScalarE exp(-ln(Z)); b_v folded into the dense bias on the
    host (sum(probs)==1).
  PSUM: one shared [128,1024] ring (2 bufs, 4 banks) carries qk-pairs
  and score pairs; a [128,512] ring (4 bufs) holds v-groups, ctx
  accumulators, Z and dense outputs.
"""

import math
import os
import sys
import types

import numpy as np
import ml_dtypes

import concourse.bass as bass
import concourse.mybir as mybir
import concourse.tile as tile
from concourse.bass import ts, ds
from concourse.bass_utils import run_bass_kernel_spmd

try:
    import orjson as _json_mod

    def _jloads(b):
        return _json_mod.loads(b)

    def _jdumps(o):
        return _json_mod.dumps(o)
except ImportError:  # pragma: no cover
    import json as _json_mod

    def _jloads(b):
        return _json_mod.loads(b)

    def _jdumps(o):
        return _json_mod.dumps(o).encode()

N_CORES = 8
SEQ, BATCH, HIDDEN, HEADS = 2048, 2, 2048, 16
HD = HIDDEN // HEADS          # 128
T = SEQ * BATCH               # 4096 tokens, batch-major: t = b*SEQ + s
P = 128
TBLK = 512                    # token block (free-dim tile)
NTB = T // TBLK               # 8
KO = HIDDEN // P              # 16 k-tiles over hidden
TOK_SHARD = T // N_CORES      # 512 tokens per core for the output
B0_SH = 256                   # per-core b0 token shard
B1_SH = 128                   # per-core b1 token shard per half
SCALE = 1.0 / math.sqrt(HD)

BF16 = mybir.dt.bfloat16
F32 = mybir.dt.float32

_last_exec_time_ns = None
_last_res = None


# ----------------------------------------------------------------------------
# Workaround: this walrus build accepts only ONE sync-wait per instruction.
# Hoist extra on_wait entries onto single-wait EventSemaphore instructions
# inserted just before the owner (same engine => same program order, so the
# semantics are identical).
# ----------------------------------------------------------------------------
def _split_multiwait(bir: dict) -> dict:
    ctr = 0
    for fn in bir.get("functions", []):
        for blk in fn.get("blocks", []):
            insts = blk.get("instructions")
            if not insts:
                continue
            new_insts = []
            changed = False
            for inst in insts:
                si = inst.get("sync_info")
                ow = (si or {}).get("on_wait") or []
                if len(ow) > 1:
                    changed = True
                    for w in ow[:-1]:
                        ctr += 1
                        new_insts.append(
                            {
                                "debug": inst.get("debug", 0),
                                "engine": inst["engine"],
                                "ins": [],
                                "name": f"{inst['name']}-mw{ctr}",
                                "opcode": "EventSemaphore",
                                "outs": [],
                                "sync_info": {"on_update": [], "on_wait": [w]},
                            }
                        )
                    si["on_wait"] = [ow[-1]]
                new_insts.append(inst)
            if changed:
                blk["instructions"] = new_insts
    return bir


def _patch_bass(nc):
    if getattr(nc, "_waitfix_patched", False):
        return nc
    orig = nc.to_json_bytes

    def patched():
        return _jdumps(_split_multiwait(_jloads(orig())))

    nc.to_json_bytes = patched
    nc._waitfix_patched = True
    return nc


def _install_ntff_hook():
    """Recreate antenv.axon_hooks if the image lacks it (needed for trace=True)."""
    try:
        from antenv.axon_hooks import get_axon_ntff_profile_hook  # noqa: F401
        return True
    except ImportError:
        pass
    try:
        from trn_agent_boot.trn_boot import _ntff_profile_via_ctypes

        hook = _ntff_profile_via_ctypes("/opt/axon/libaxon_pjrt.so")
        if hook is None:
            return False
        mod = types.ModuleType("antenv.axon_hooks")
        mod._hook = hook
        mod.get_axon_ntff_profile_hook = lambda: mod._hook
        mod.set_axon_ntff_profile_hook = lambda h: setattr(mod, "_hook", h)
        sys.modules["antenv.axon_hooks"] = mod
        import antenv

        antenv.axon_hooks = mod
        return True
    except Exception:
        return False


# ----------------------------------------------------------------------------
# Device graph (SPMD: same graph on all 8 cores)
# ----------------------------------------------------------------------------
def _build():
    nc = bass.Bass()

    xT = nc.declare_dram_parameter("xT", [HIDDEN, T], BF16, isOutput=False)
    wqk = nc.declare_dram_parameter("wqk", [HIDDEN, 4 * P], BF16, isOutput=False)
    wv = nc.declare_dram_parameter("wv", [HIDDEN, 2 * P], BF16, isOutput=False)
    # wd pre-tiled on the host as [ot, p, ki, ocol] so each per-ot load is
    # one contiguous 4KB-per-partition DMA
    wd = nc.declare_dram_parameter("wd", [KO, P, KO, P], BF16, isOutput=False)
    bqk = nc.declare_dram_parameter("bqk", [P, 4], F32, isOutput=False)
    bd = nc.declare_dram_parameter("bd", [P, KO], F32, isOutput=False)  # + Wd@bv
    out = nc.declare_dram_parameter("out", [HIDDEN, TOK_SHARD], F32, isOutput=True)

    xT_r = xT.rearrange("(ko p) t -> p ko t", p=P)
    wqk_r = wqk.rearrange("(ko p) c -> p ko c", p=P)
    wv_r = wv.rearrange("(ko p) c -> p ko c", p=P)

    Exp = mybir.ActivationFunctionType.Exp
    Ln = mybir.ActivationFunctionType.Ln
    Ident = mybir.ActivationFunctionType.Identity

    with tile.TileContext(nc) as tc:
        with (
            tc.tile_pool(name="const", bufs=1) as pc,
            tc.tile_pool(name="xs", bufs=2) as px,
            tc.tile_pool(name="es", bufs=14) as pe,
            tc.tile_pool(name="esum", bufs=3) as pE,
            tc.tile_pool(name="fs", bufs=3) as pf,
            tc.tile_pool(name="wds", bufs=7) as pwd,
            tc.tile_pool(name="outs", bufs=3) as po,
            tc.tile_pool(name="ps_ring", bufs=2, space="PSUM") as pps,
            tc.tile_pool(name="ps_acc", bufs=4, space="PSUM") as pacc,
            tc.tile_pool(name="dram", bufs=1, space="DRAM") as pdram,
        ):
            # ---- constants ----
            # preamble rides three queues in parallel: wqk on sync, x0 on
            # scalar, wv on gpsimd — the tb0 ko-major qk loop consumes
            # wqk[ko]+x0[ko] at ~250GB/s aggregate, which one queue alone
            # cannot sustain.
            wqk_sb = pc.tile([P, KO, 4 * P], BF16)
            x0_sb = px.tile([P, KO, TBLK], BF16, tag="x")
            for lo, n in [(0, 1), (1, 1), (2, 1), (3, 1), (4, 2), (6, 2),
                          (8, 4), (12, 4)]:
                nc.sync.dma_start(
                    wqk_sb[:, ds(lo, n), :], wqk_r[:, ds(lo, n), :]
                )
                nc.sync.dma_start(
                    x0_sb[:, ds(lo, n), :], xT_r[:, ds(lo, n), ts(0, TBLK)]
                )
            wv_sb = pc.tile([P, KO, 2 * P], BF16)
            for ko4 in range(4):
                nc.gpsimd.dma_start(
                    wv_sb[:, ts(ko4, 4), :], wv_r[:, ts(ko4, 4), :]
                )
            bqk_sb = pc.tile([P, 4], F32)
            nc.sync.dma_start(bqk_sb[:], bqk[:])
            bd_sb = pc.tile([P, KO], F32)
            nc.sync.dma_start(bd_sb[:], bd[:])

            ones_sb = pc.tile([P, P], BF16)
            nc.vector.memset(ones_sb[:], 1.0)
            # triangular mask [sk, sq_local]: keep where sq >= sk
            tri_sb = pc.tile([P, P], BF16)
            nc.vector.memset(tri_sb[:], 1.0)
            nc.gpsimd.affine_select(
                out=tri_sb[:],
                in_=tri_sb[:],
                compare_op=mybir.AluOpType.is_ge,
                fill=0.0,
                base=0,
                pattern=[[1, P]],
                channel_multiplier=-1,
            )

            qk_sb = pc.tile([P, 4, T], BF16)   # [d, (q_h0,k_h0,q_h1,k_h1), tok]
            v_sb = pc.tile([P, T // P, 2 * P], BF16)  # [tok_in_tile, tile, (v0,v1)]

            # a2a staging; shard content is [p, head, col] so one DMA per
            # destination carries BOTH heads (the two epilogues of a
            # section share one staging tile) — half the writes, double
            # the line size, and the write chain gates the collectives.
            a2a_inA = pdram.tile([N_CORES, P, 2, B0_SH], BF16,
                                 name="a2a_inA", tag="a2a_inA")
            a2a_outA = pdram.tile([N_CORES, P, 2, B0_SH], BF16,
                                  name="a2a_outA", tag="a2a_outA")
            # b1 ships in three waves so every dense input is on-core well
            # before its pass starts, whatever the fabric latency: B1 =
            # query-blocks 0-1 (128-token shards, trigger 2 sections before
            # the end), B2 = block 2 (64, trigger 1 section before), B3 =
            # block 3 (64, trigger at the tail under ~50us of dense cover).
            BW = [128, 64, 64]
            a2a_inB = [
                pdram.tile([N_CORES, P, 2, BW[k]], BF16,
                           name=f"a2a_inB{k}", tag=f"a2a_inB{k}")
                for k in range(3)
            ]
            a2a_outB = [
                pdram.tile([N_CORES, P, 2, BW[k]], BF16,
                           name=f"a2a_outB{k}", tag=f"a2a_outB{k}")
                for k in range(3)
            ]
            # [d, global head, tok] gathered ctx for this core's shard:
            # cols 0:256 b0, 256:384 b1-qb01, 384:448 qb2, 448:512 qb3
            ctxT = pc.tile([P, HEADS, TOK_SHARD], BF16, name="ctxT")
            BOFF = [B0_SH, B0_SH + 128, B0_SH + 192]

            # collective triggers ride gpsimd (and ONLY the triggers: the
            # gpsimd SWDGE costs ~3.3us per dma op, serially, so any loads
            # placed there both run late and block later triggers).  The
            # gathered-ctx loads ride the sync HWDGE queue instead, emitted
            # at a point where their gating collective is already complete
            # so they never head-of-line-block the x stream.
            def emit_a2a(which):
                if which == "A":
                    ins, outs = a2a_inA, a2a_outA
                else:
                    ins, outs = a2a_inB[int(which)], a2a_outB[int(which)]
                nc.gpsimd.collective_compute(
                    "AllToAll",
                    mybir.AluOpType.bypass,
                    replica_groups=[list(range(N_CORES))],
                    ins=[ins[:].opt()],
                    outs=[outs[:].opt()],
                )

            def emit_ctx_loads(which):
                if which == "A":
                    src, c0, w = a2a_outA, 0, B0_SH
                else:
                    k = int(which)
                    src, c0, w = a2a_outB[k], BOFF[k], BW[k]
                for i in range(N_CORES):
                    for h in range(2):
                        nc.sync.dma_start(
                            ctxT[:, 2 * i + h, ds(c0, w)],
                            src[i, :, h, :],
                        )

            # ------------------------------------------------------------
            # attention block emission.  Returns (s_units, c_units, state):
            # score units (diag packs first, then off-diag pairs, then the
            # E finisher) and ctx-matmul units.  Row sums accumulate into
            # E on the DVE ("a-mode"); the Z ones-matmul runs in the
            # epilogue.
            # ------------------------------------------------------------
            def make_block(h, b, qb):
                base = b * SEQ
                q0 = base + qb * TBLK
                st = {}

                def k_ap(kt):
                    return qk_sb[:, 2 * h + 1, ds(base + kt * P, P)]

                s_units = []
                c_units = []

                def sA():
                    st["Et"] = pE.tile([P, TBLK], F32, tag="Ef", name="Et")
                    psA = pps.tile([P, 2 * TBLK], F32, tag="ps", name="psA")
                    st["psA"] = psA
                    nc.tensor.matmul(
                        psA[:, 0:TBLK], lhsT=k_ap(4 * qb + 0),
                        rhs=qk_sb[:, 2 * h, ds(q0, TBLK)], start=True, stop=True,
                    )
                    nc.tensor.matmul(
                        psA[:, ds(TBLK, 384)], lhsT=k_ap(4 * qb + 1),
                        rhs=qk_sb[:, 2 * h, ds(q0 + P, 384)], start=True, stop=True,
                    )
                s_units.append(sA)

                def sAe():
                    eA = pe.tile([P, 2 * TBLK], BF16, tag="e", name="eA")
                    st["eA"] = eA
                    nc.scalar.activation(
                        eA[:, 0:TBLK + 384], st["psA"][:, 0:TBLK + 384],
                        Exp, scale=SCALE,
                    )
                    nc.vector.tensor_mul(eA[:, 0:P], eA[:, 0:P], tri_sb[:])
                    nc.vector.tensor_mul(
                        eA[:, ds(TBLK, P)], eA[:, ds(TBLK, P)], tri_sb[:]
                    )
                    if qb == 0:
                        nc.vector.tensor_copy(st["Et"][:], eA[:, 0:TBLK])
                s_units.append(sAe)

                def sB():
                    psB = pps.tile([P, 2 * TBLK], F32, tag="ps", name="psB")
                    st["psB"] = psB
                    nc.tensor.matmul(
                        psB[:, 0:256], lhsT=k_ap(4 * qb + 2),
                        rhs=qk_sb[:, 2 * h, ds(q0 + 2 * P, 256)],
                        start=True, stop=True,
                    )
                    nc.tensor.matmul(
                        psB[:, ds(TBLK, P)], lhsT=k_ap(4 * qb + 3),
                        rhs=qk_sb[:, 2 * h, ds(q0 + 3 * P, P)],
                        start=True, stop=True,
                    )
                s_units.append(sB)

                def sBe():
                    eB = pe.tile([P, 2 * TBLK], BF16, tag="e", name="eB")
                    st["eB"] = eB
                    nc.scalar.activation(
                        eB[:, 0:TBLK + P], st["psB"][:, 0:TBLK + P],
                        Exp, scale=SCALE,
                    )
                    nc.vector.tensor_mul(eB[:, 0:P], eB[:, 0:P], tri_sb[:])
                    nc.vector.tensor_mul(
                        eB[:, ds(TBLK, P)], eB[:, ds(TBLK, P)], tri_sb[:]
                    )
                s_units.append(sBe)

                for pr in range(2 * qb):
                    def s_od(pr=pr):
                        ps_s = pps.tile([P, 2 * TBLK], F32, tag="ps", name="ps_s")
                        for half in range(2):
                            kt = 2 * pr + half
                            nc.tensor.matmul(
                                ps_s[:, ts(half, TBLK)],
                                lhsT=k_ap(kt),
                                rhs=qk_sb[:, 2 * h, ds(q0, TBLK)],
                                start=True, stop=True,
                            )
                        e_od = pe.tile([P, 2 * TBLK], BF16, tag="e", name="e_od")
                        st[f"e{pr}"] = e_od
                        nc.scalar.activation(e_od[:], ps_s[:], Exp, scale=SCALE)
                        Et = st["Et"]
                        if pr == 0:
                            nc.vector.tensor_add(
                                Et[:], e_od[:, 0:TBLK], e_od[:, ts(1, TBLK)]
                            )
                        else:
                            nc.vector.tensor_add(
                                Et[:], Et[:], e_od[:, 0:TBLK]
                            )
                            nc.vector.tensor_add(
                                Et[:], Et[:], e_od[:, ts(1, TBLK)]
                            )
                    s_units.append(s_od)

                def s_fin():
                    Et, eA, eB = st["Et"], st["eA"], st["eB"]
                    if qb > 0:
                        nc.vector.tensor_add(Et[:], Et[:], eA[:, 0:TBLK])
                    nc.vector.tensor_add(
                        Et[:, ds(P, 384)], Et[:, ds(P, 384)], eA[:, ds(TBLK, 384)]
                    )
                    nc.vector.tensor_add(
                        Et[:, ds(2 * P, 256)], Et[:, ds(2 * P, 256)], eB[:, 0:256]
                    )
                    nc.vector.tensor_add(
                        Et[:, ds(3 * P, P)], Et[:, ds(3 * P, P)], eB[:, ds(TBLK, P)]
                    )
                    E_bf = pE.tile([P, TBLK], BF16, tag="Eb", name="E_bf")
                    st["E_bf"] = E_bf
                    nc.vector.tensor_copy(E_bf[:], Et[:])
                s_units.append(s_fin)

                n_cmm = 4 * qb + 4
                cmm = [0]

                def c_mm(e_t, kt, e_col, w, q_off):
                    if cmm[0] == 0:
                        st["ps_ctx"] = pacc.tile(
                            [P, TBLK], F32, tag="acc", name="ps_ctx"
                        )
                    nc.tensor.matmul(
                        st["ps_ctx"][:, ds(q_off, w)],
                        lhsT=v_sb[:, b * (SEQ // P) + kt, ts(h, P)],
                        rhs=e_t[:, ds(e_col, w)],
                        start=(cmm[0] == 0),
                        stop=(cmm[0] == n_cmm - 1),
                    )
                    cmm[0] += 1

                def cA():
                    c_mm(st["eA"], 4 * qb + 0, 0, TBLK, 0)
                    c_mm(st["eA"], 4 * qb + 1, TBLK, 384, P)
                c_units.append(cA)

                def cB():
                    c_mm(st["eB"], 4 * qb + 2, 0, 256, 2 * P)
                    c_mm(st["eB"], 4 * qb + 3, TBLK, P, 3 * P)
                c_units.append(cB)

                for pr in range(2 * qb):
                    def c_od(pr=pr):
                        e_od = st[f"e{pr}"]
                        c_mm(e_od, 2 * pr, 0, TBLK, 0)
                        c_mm(e_od, 2 * pr + 1, TBLK, TBLK, 0)
                    c_units.append(c_od)

                def get_ep():
                    return (st["ps_ctx"], st["E_bf"], h, b, qb)

                st["get_ep"] = get_ep
                return s_units, c_units, st

            def emit_epilogue_pair(eps, z_in_pps=False):
                # both heads of one (b, qb) block: softmax-normalize into a
                # shared [p, head, col] staging tile, then one write per dst
                ctxb = pf.tile([P, 2, TBLK], BF16, tag="ctxb", name="ctxb")
                b, qb = eps[0][3], eps[0][4]
                for h, (ps_ctx, E_bf, _h, _b, _qb) in enumerate(eps):
                    if z_in_pps:
                        ps_z = pps.tile([P, 2 * TBLK], F32, tag="ps",
                                        name="ps_z")
                        ps_z = ps_z[:, 0:TBLK]
                    else:
                        ps_z = pacc.tile([P, TBLK], F32, tag="acc",
                                         name="ps_z")
                        ps_z = ps_z[:]
                    nc.tensor.matmul(
                        ps_z, lhsT=ones_sb[:], rhs=E_bf[:],
                        start=True, stop=True,
                    )
                    lnz = pf.tile([P, TBLK], F32, tag="lnz", name="lnz")
                    nc.scalar.activation(lnz[:], ps_z, Ln, scale=1.0)
                    recip = pf.tile([P, TBLK], F32, tag="recip", name="recip")
                    nc.scalar.activation(recip[:], lnz[:], Exp, scale=-1.0)
                    nc.vector.tensor_mul(ctxb[:, h, :], ps_ctx[:], recip[:])
                if b == 0:
                    for j in range(2):
                        nc.sync.dma_start(
                            a2a_inA[2 * qb + j, :, :, :],
                            ctxb[:, :, ts(j, B0_SH)],
                        )
                elif qb < 2:
                    for j in range(4):
                        nc.sync.dma_start(
                            a2a_inB[0][4 * qb + j, :, :, :],
                            ctxb[:, :, ts(j, BW[0])],
                        )
                else:
                    for dst in range(N_CORES):
                        nc.sync.dma_start(
                            a2a_inB[qb - 1][dst, :, :, :],
                            ctxb[:, :, ts(dst, 64)],
                        )

            # ---- 8 sections: QKV(tb) + attention (h0,h1) at (b,qb)=tb ----
            pend = []          # epilogues carried into the next section
            wd_pre = []        # dense wd tiles prefetched during tb7
            for tb in range(NTB):
                b, qb = tb // 4, tb % 4
                if tb == NTB - 1:
                    # a2a#A completed long ago: its ctx loads go on sync
                    # here (no head-of-line blocking — the last x load was
                    # emitted above), then the first dense wd tiles, then
                    # the B1 ctx loads (gated on a2a#B1, done by ~2
                    # sections later at the latest).
                    emit_ctx_loads("A")
                    for j in range(5):
                        w_t = pwd.tile([P, KO, P], BF16, tag="wd")
                        nc.sync.dma_start(w_t[:], wd[j, :, :, :])
                        wd_pre.append(w_t)
                    emit_ctx_loads("0")
                if tb == 0:
                    x_sb = x0_sb
                else:
                    x_sb = px.tile([P, KO, TBLK], BF16, tag="x")
                    for ko4 in range(4):
                        nc.sync.dma_start(
                            x_sb[:, ts(ko4, 4), :],
                            xT_r[:, ts(ko4, 4), ts(tb, TBLK)],
                        )

                def qk_pair(cp):
                    ps_qk = pps.tile([P, 2 * TBLK], F32, tag="ps", name="ps_qk")
                    for half in range(2):
                        ct = 2 * cp + half
                        for ko in range(KO):
                            nc.tensor.matmul(
                                ps_qk[:, ts(half, TBLK)],
                                lhsT=wqk_sb[:, ko, ts(ct, P)],
                                rhs=x_sb[:, ko, :],
                                start=(ko == 0),
                                stop=(ko == KO - 1),
                            )
                    for half in range(2):
                        ct = 2 * cp + half
                        nc.scalar.activation(
                            qk_sb[:, ct, ts(tb, TBLK)],
                            ps_qk[:, ts(half, TBLK)], Ident,
                            bias=bqk_sb[:, ct : ct + 1], scale=1.0,
                        )

                def v_group(vt):
                    ps_v = pacc.tile([P, 2 * P], F32, tag="acc", name="ps_v")
                    for ko in range(KO):
                        nc.tensor.matmul(
                            ps_v[:],
                            lhsT=x_sb[:, ko, ts(vt, P)],
                            rhs=wv_sb[:, ko, :],
                            start=(ko == 0),
                            stop=(ko == KO - 1),
                        )
                    nc.scalar.copy(v_sb[:, tb * (TBLK // P) + vt, :], ps_v[:])

                susA, cusA, stA = make_block(0, b, qb)
                susB, cusB, stB = make_block(1, b, qb)

                if tb == 0:
                    # cold start is DMA-bound: ko-major qk (4 concurrent
                    # PSUM accumulations) consumes only 256KB of fresh
                    # wqk+x per 1.05us of matmuls, which the HBM stream
                    # can actually sustain -- ct-major would idle the PE
                    # for ~8us waiting on the x chunks.
                    ps01 = pps.tile([P, 2 * TBLK], F32, tag="ps", name="ps01")
                    ps23 = pps.tile([P, 2 * TBLK], F32, tag="ps", name="ps23")
                    for ko in range(KO):
                        for pst, cp in ((ps01, 0), (ps23, 1)):
                            for half in range(2):
                                ct = 2 * cp + half
                                nc.tensor.matmul(
                                    pst[:, ts(half, TBLK)],
                                    lhsT=wqk_sb[:, ko, ts(ct, P)],
                                    rhs=x_sb[:, ko, :],
                                    start=(ko == 0),
                                    stop=(ko == KO - 1),
                                )
                    for pst, cp in ((ps01, 0), (ps23, 1)):
                        for half in range(2):
                            ct = 2 * cp + half
                            nc.scalar.activation(
                                qk_sb[:, ct, ts(tb, TBLK)],
                                pst[:, ts(half, TBLK)], Ident,
                                bias=bqk_sb[:, ct : ct + 1], scale=1.0,
                            )
                    v_group(0)
                    susA[0]()     # A.sA
                    v_group(1)
                    susA[1]()     # A.sAe
                    susA[2]()     # A.sB
                    v_group(2)
                    susA[3]()     # A.sBe
                    susB[0]()     # B.sA
                    v_group(3)
                    susB[1]()     # B.sAe
                    susB[2]()     # B.sB
                    susB[3]()     # B.sBe
                    susA[4]()     # A.s_fin
                    cusA[0]()     # A.cA
                    cusA[1]()     # A.cB
                    susB[4]()     # B.s_fin
                    cusB[0]()     # B.cA
                    cusB[1]()     # B.cB
                else:
                    # head: qk pairs + prev-section epilogues + diag blocks
                    qk_pair(0)
                    v_group(0)
                    susA[0]()     # A.sA
                    if pend:
                        emit_epilogue_pair(pend)
                        prev_tb = tb - 1
                        pend = []
                        if prev_tb == 3:
                            emit_a2a("A")
                        elif prev_tb == 5:
                            emit_a2a("0")
                        elif prev_tb == 6:
                            emit_a2a("1")
                    susA[1]()     # A.sAe
                    qk_pair(1)
                    susA[2]()     # A.sB
                    susA[3]()     # A.sBe
                    susB[0]()     # B.sA
                    susB[1]()     # B.sAe
                    susB[2]()     # B.sB
                    susB[3]()     # B.sBe
                    v_group(1)
                    if qb == 0:
                        susA[4]()     # fins
                        susB[4]()
                        v_group(2)
                        cusA[0]()
                        cusB[0]()
                        v_group(3)
                        cusA[1]()
                        cusB[1]()
                    else:
                        # tail: alternate od pairs between the two blocks,
                        # sprinkle remaining v-groups, ctx units lag 1 pair
                        ods = list(range(2 * qb))
                        ciA = ciB = 0
                        for k, pr in enumerate(ods):
                            susA[4 + pr]()
                            susB[4 + pr]()
                            if k == 0:
                                v_group(2)
                            elif k == 1:
                                v_group(3)
                            if k >= 1:
                                if ciA < len(cusA):
                                    cusA[ciA]()
                                    ciA += 1
                                if ciB < len(cusB):
                                    cusB[ciB]()
                                    ciB += 1
                        susA[4 + 2 * qb]()   # A.s_fin
                        susB[4 + 2 * qb]()   # B.s_fin
                        while ciA < len(cusA) or ciB < len(cusB):
                            if ciA < len(cusA):
                                cusA[ciA]()
                                ciA += 1
                            if ciB < len(cusB):
                                cusB[ciB]()
                                ciB += 1
                pend = [stA["get_ep"](), stB["get_ep"]()]

            # ---- dense projection, two disjoint-token passes ----
            # wd loads ride the scalar queue (x is done by then, out
            # writes are small); the collective-gated ctxT loads stay on
            # gpsimd so wd streaming never queues behind a collective
            # wait, and sync stays clear for the a2a_in writes.
            wd_q = wd_pre    # rolling prefetch queue, distance 4

            def wd_prefetch(src_ot):
                w_t = pwd.tile([P, KO, P], BF16, tag="wd")
                nc.scalar.dma_start(w_t[:], wd[src_ot, :, :, :])
                wd_q.append(w_t)

            def dense_ot(col0, ncol, ot, pre_ot=None):
                wd_sb = wd_q.pop(0)
                if pre_ot is not None:
                    wd_prefetch(pre_ot)
                ps_o = pacc.tile([P, ncol], F32, tag="acc", name="ps_o")
                for i in range(KO):
                    nc.tensor.matmul(
                        ps_o[:],
                        lhsT=wd_sb[:, i, :],
                        rhs=ctxT[:, i, ds(col0, ncol)],
                        start=(i == 0),
                        stop=(i == KO - 1),
                    )
                out_sb = po.tile([P, ncol], F32, tag="osb")
                nc.scalar.activation(
                    out_sb[:], ps_o[:], Ident,
                    bias=bd_sb[:, ot : ot + 1], scale=1.0,
                )
                nc.scalar.dma_start(out[ts(ot, P), ds(col0, ncol)], out_sb[:])

            # ---- tail: last epilogues close a2a#B2 between the first
            # dense iterations, so the PE covers their ACT/DVE chains and
            # the collective fires ~10us earlier than a pure
            # epilogues-then-dense order would allow.
            # use-sequence: pass A ots 0..15, then pass B ots 0..15; the
            # prefetch for use u+4 is issued at use u (wd_pre seeds 0..3)
            def pre_idx(u):
                n = u + 5
                if n < KO:
                    return n
                if n < 2 * KO:
                    return n - KO
                return None

            # PE order: dense0 first (its PSUM slot is free and operands
            # long-loaded, so it bridges the ACT/DVE epilogue tail), then
            # both epilogues back-to-back: E_bf has drained from the DVE
            # so the Z matmuls barely stall the PE, and the a2a#B3 chain
            # (one bias ahead of ln on ACT) fires ~6us after the last
            # attention work, under ~34us of pass-A cover.
            dense_ot(0, B0_SH, 0, pre_idx(0))
            emit_epilogue_pair(pend, z_in_pps=True)
            pend = []
            emit_a2a("2")
            for ot in range(1, KO):
                dense_ot(0, B0_SH, ot, pre_idx(ot))
            emit_ctx_loads("1")   # gated on a2a#B2, done by now
            emit_ctx_loads("2")   # gated on a2a#B3; nothing queues behind
            for ot in range(KO):
                dense_ot(B0_SH, B0_SH, ot, pre_idx(KO + ot))

    _patch_bass(nc)
    return nc


_cached_nc = None


def _get_nc():
    global _cached_nc
    if _cached_nc is None:
        _cached_nc = _build()
    return _cached_nc


# ----------------------------------------------------------------------------
# Host entry point
# ----------------------------------------------------------------------------
def kernel(x, mask, w_qkv, b_qkv, w_dense, b_dense):
    global _last_exec_time_ns, _last_res
    x = np.asarray(x, dtype=np.float32)
    w_qkv = np.asarray(w_qkv, dtype=np.float32)
    b_qkv = np.asarray(b_qkv, dtype=np.float32)
    w_dense = np.asarray(w_dense, dtype=np.float32)
    b_dense = np.asarray(b_dense, dtype=np.float32)

    bf16 = ml_dtypes.bfloat16
    # tokens batch-major: t = b*SEQ + s
    xT = np.ascontiguousarray(
        x.transpose(1, 0, 2).reshape(T, HIDDEN).T
    ).astype(bf16)
    # wd4[ot, p, ki, ocol] = w_dense.T[ki*128+p, ot*128+ocol]
    wd4 = np.ascontiguousarray(
        w_dense.T.reshape(KO, P, KO, P).transpose(2, 1, 0, 3)
    ).astype(bf16)

    # fold b_v into the dense bias: out = Wd@(probs@v) + (bd + Wd@bv)
    bv_full = np.empty(HIDDEN, dtype=np.float64)
    for hh in range(HEADS):
        bv_full[hh * HD:(hh + 1) * HD] = b_qkv[hh * 384 + 256: hh * 384 + 384]
    bd_eff = (
        b_dense.astype(np.float64) + w_dense.astype(np.float64) @ bv_full
    ).astype(np.float32)
    bd_host = np.ascontiguousarray(bd_eff.reshape(KO, P).T)

    in_maps = []
    for c in range(N_CORES):
        h0, h1 = 2 * c, 2 * c + 1
        qk_rows = np.concatenate(
            [
                np.arange(h0 * 384, h0 * 384 + 128),        # q_h0
                np.arange(h0 * 384 + 128, h0 * 384 + 256),  # k_h0
                np.arange(h1 * 384, h1 * 384 + 128),        # q_h1
                np.arange(h1 * 384 + 128, h1 * 384 + 256),  # k_h1
            ]
        )
        v_rows = np.concatenate(
            [
                np.arange(h0 * 384 + 256, h0 * 384 + 384),  # v_h0
                np.arange(h1 * 384 + 256, h1 * 384 + 384),  # v_h1
            ]
        )
        in_maps.append(
            {
                "xT": xT,
                "wqk": np.ascontiguousarray(w_qkv[qk_rows].T).astype(bf16),
                "wv": np.ascontiguousarray(w_qkv[v_rows].T).astype(bf16),
                "wd": wd4,
                "bqk": np.ascontiguousarray(b_qkv[qk_rows].reshape(4, P).T),
                "bd": bd_host,
            }
        )

    nc = _get_nc()
    trace = bool(int(os.environ.get("KERNEL_TRACE", "0")))
    if trace:
        trace = _install_ntff_hook()
    res = run_bass_kernel_spmd(
        nc, in_maps, core_ids=list(range(N_CORES)), trace=trace
    )
    _last_exec_time_ns = res.exec_time_ns
    _last_res = res

    # outs[c]["out"] is out^T [HIDDEN, 512]: cols 0:256 -> b0 tokens
    # [256c, 256c+256); cols 256:384 -> b1 tokens [128c, +128); cols
    # 384:512 -> b1 tokens [1024+128c, +128)   (b1 tokens offset 2048)
    full_T = np.empty((HIDDEN, T), dtype=np.float32)
    for c in range(N_CORES):
        o = res.results[c]["out"]
        full_T[:, 256 * c: 256 * c + 256] = o[:, 0:256]
        full_T[:, 2048 + 128 * c: 2048 + 128 * c + 128] = o[:, 256:384]
        full_T[:, 3072 + 64 * c: 3072 + 64 * c + 64] = o[:, 384:448]
        full_T[:, 3584 + 64 * c: 3584 + 64 * c + 64] = o[:, 448:512]
    full = full_T.T  # [T, HIDDEN], batch-major tokens
    return np.ascontiguousarray(
        full.reshape(BATCH, SEQ, HIDDEN).transpose(1, 0, 2)
    ).astype(np.float32)


def last_exec_time_ns():
    return _last_exec_time_ns
